# revision 1
# baseline (speedup 1.0000x reference)
"""GATv2 attention-weights kernel for 8 Trainium2 NeuronCores.

Problem (per full input):
    q: (2, 8, 384, 64) f32, k: (2, 8, 384, 64) f32,
    attention: (1, 8, 1, 1, 64) f32, mask: (2, 8, 384, 384) bool
    scores[b,h,i,j] = sum_d silu(q[b,h,i,d] + k[b,h,j,d]) * attention[h,d]
    out = softmax over j with mask (-inf before, 0 after)

Sharding: data-parallel over the 16 (b,h) pairs, 2 per core.

Per-core device pipeline (raw bass, explicit semaphores; "jj,d" packing =
two j columns share the 128 partitions, d=64 each half):
    - DVE builds T[(jj,d), i] = qT_rep + k_pair  (per-partition-scalar add,
      2x perf mode) for G j-pairs per group
    - ACT computes silu IN PLACE on T (ACT is the throughput floor:
      LQ*LK*D silu evaluations per (b,h) at 128 lanes / 1.2 GHz)
    - PE reduces over d with the `a` vector folded into the weights:
      matmul(lhsT=T_block[(jj,d), i_block], rhs=a2[(jj,d), 2]) ->
      scores[i_block, j_pair] land un-transposed in PSUM (6 banks hold all
      scores for both (b,h))
    - Masked softmax over the free dim afterwards (one activation-table
      switch to Exp for the whole kernel): fused (mask*-1e30)+scores on DVE,
      exp with fused row-sum (accum_out) on ACT, reciprocal + scale on DVE.
      No per-row max: scores are bounded (|s| < 8), exp cannot overflow.
"""

import numpy as np
from contextlib import ExitStack

import concourse.bass as bass
from concourse import mybir
from concourse.bass_utils import run_bass_kernel_spmd

B, H, LQ, LK, D = 2, 8, 384, 384, 64
NCORES = 8
NBH = (B * H) // NCORES        # 2 (b,h) pairs per core
NPAIR = LK // 2                # 192 j-pairs
# j-pairs per silu group: bh0 ramps up so the pipeline fills fast, then
# steady-state groups are large to amortize the ACT per-instruction overhead
GROUPS_BH = [[2, 4, 8, 8, 12, 16, 20, 26, 32, 36, 28], [36, 36, 36, 36, 36, 12]]
assert all(sum(g) == NPAIR for g in GROUPS_BH)
GMAX = max(max(g) for g in GROUPS_BH)
# flattened per-rep schedule: (bh, size, pair_offset)
GLIST = [(bh, s, off)
         for bh in range(2)
         for s, off in zip(GROUPS_BH[bh],
                           [sum(GROUPS_BH[bh][:i])
                            for i in range(len(GROUPS_BH[bh]))])]
G0 = len(GROUPS_BH[0])         # groups in bh0
GG = len(GLIST)                # global groups per rep
NIB = LQ // 128                # 3 i-blocks
NSM = NBH * NIB                # 6 softmax tiles
QKA = LQ + NPAIR + 2           # packed constants width (per partition, f32)

_f32 = mybir.dt.float32
_u8 = mybir.dt.uint8

_built = None  # cache across calls


def _build(reps=1):
    # reps > 1 unrolls the whole computation N times inside one program
    # (used only for steady-state timing; the grading path uses reps=1).
    AF = mybir.ActivationFunctionType
    Alu = mybir.AluOpType

    nc = bass.Bass("TRN2", target_bir_lowering=False, debug=False,
                   num_devices=NCORES)

    qka_d = nc.dram_tensor("qka", [NBH, 128, QKA], _f32, kind="ExternalInput").ap()
    mask_d = nc.dram_tensor("masku8", [NBH, LQ, LK], _u8, kind="ExternalInput").ap()
    w_d = nc.dram_tensor("w", [NBH, LQ, LK], _f32, kind="ExternalOutput").ap()

    qka_t = [nc.alloc_sbuf_tensor(f"qka_t{bh}", [128, QKA], _f32).ap()
             for bh in range(NBH)]
    mask_t = [nc.alloc_sbuf_tensor(f"mask_t{i}", [128, LK], _u8).ap()
              for i in range(NSM)]
    T_t = [nc.alloc_sbuf_tensor(f"T{s}", [128, GMAX * LQ], _f32).ap()
           for s in range(3)]
    E_t = [nc.alloc_sbuf_tensor(f"E{i}", [128, LK], _f32).ap()
           for i in range(NSM)]
    W_t = [nc.alloc_sbuf_tensor(f"W{i}", [128, LK], _f32).ap()
           for i in range(NSM)]
    sums_t = [nc.alloc_sbuf_tensor(f"sums{i}", [128, 1], _f32).ap()
              for i in range(NSM)]
    r_t = [nc.alloc_sbuf_tensor(f"r{i}", [128, 1], _f32).ap()
           for i in range(NSM)]
    sc_t = [nc.alloc_psum_tensor(f"sc{i}", [128, LK], _f32).ap()
            for i in range(NSM)]

    def qtrep(bh):
        return qka_t[bh][:, 0:LQ]

    def kpair(bh, p):
        return qka_t[bh][:, LQ + p:LQ + p + 1]

    def a2(bh):
        return qka_t[bh][:, LQ + NPAIR:LQ + NPAIR + 2]

    with ExitStack() as ctx:
        s_qka = [ctx.enter_context(nc.semaphore(f"s_qka{bh}")) for bh in range(NBH)]
        s_qk0b = ctx.enter_context(nc.semaphore("s_qk0b"))
        s_qk0c = ctx.enter_context(nc.semaphore("s_qk0c"))
        s_mask = ctx.enter_context(nc.semaphore("s_mask"))
        s_T = ctx.enter_context(nc.semaphore("s_T"))
        s_S = ctx.enter_context(nc.semaphore("s_S"))
        s_pe = ctx.enter_context(nc.semaphore("s_pe"))
        s_scm = ctx.enter_context(nc.semaphore("s_scm"))
        s_E = ctx.enter_context(nc.semaphore("s_E"))
        s_W = ctx.enter_context(nc.semaphore("s_W"))
        s_wsp = ctx.enter_context(nc.semaphore("s_wsp"))
        s_wact = ctx.enter_context(nc.semaphore("s_wact"))
        block = ctx.enter_context(nc.Block())

        CH0 = LQ + 16  # first chunk: qtrep + 16 kpairs (gates group 0..2)

        @block.sync
        def _(sp):
            # bh0 leading chunk first (everything upstream gates on it),
            # a2 rides in the same chunk via a second tiny DMA on the queue
            sp.dma_start(out=qka_t[0][:, 0:CH0],
                         in_=qka_d[0][:, 0:CH0]).then_inc(s_qka[0], 16)
            sp.dma_start(out=qka_t[0][:, LQ + NPAIR:LQ + NPAIR + 2],
                         in_=qka_d[0][:, LQ + NPAIR:LQ + NPAIR + 2]
                         ).then_inc(s_qk0b, 16)
            sp.dma_start(out=qka_t[0][:, CH0:LQ + NPAIR],
                         in_=qka_d[0][:, CH0:LQ + NPAIR]
                         ).then_inc(s_qk0c, 16)
            sp.dma_start(out=qka_t[1], in_=qka_d[1]).then_inc(s_qka[1], 16)
            for idx in range(NSM):
                bh, ib = divmod(idx, NIB)
                sp.dma_start(out=mask_t[idx],
                             in_=mask_d[bh, ib * 128:(ib + 1) * 128, :]
                             ).then_inc(s_mask, 16)
            # output DMAs: late tiles (3,4,5) on the SP HWDGE queue,
            # early tiles (0,1,2) on ACT's HWDGE queue (see scalar block)
            for rep in range(reps):
                for idx in range(NSM // 2, NSM):
                    bh, ib = divmod(idx, NIB)
                    sp.wait_ge(s_W, rep * NSM + idx + 1)
                    sp.dma_start(out=w_d[bh, ib * 128:(ib + 1) * 128, :],
                                 in_=W_t[idx]).then_inc(s_wsp, 16)
            sp.wait_ge(s_wsp, 16 * (NSM // 2) * reps)
            sp.wait_ge(s_wact, 16 * (NSM // 2) * reps)

        @block.vector
        def _(v):
            def tbuild(v, rep, gg):
                bh, size, off = GLIST[gg]
                gi = rep * GG + gg
                if rep == 0 and bh == 1 and off == 0:
                    v.wait_ge(s_qka[1], 16)
                if gi >= 3:
                    v.wait_ge(s_pe, gi - 2)
                T = T_t[gi % 3]
                for pl in range(size):
                    p = off + pl
                    ins = v.tensor_scalar_add(
                        T[:, pl * LQ:(pl + 1) * LQ], qtrep(bh), kpair(bh, p))
                ins.then_inc(s_T, 1)

            def scm(v, rep, bh):
                # mask+scores fuse for this bh.  bh0's is emitted a few
                # groups into bh1's stream so its s_pe wait is already
                # implied and DVE never stalls at the bh boundary.
                if rep == 0 and bh == 0:
                    v.wait_ge(s_mask, 16 * NSM)
                for ib in range(NIB):
                    idx = bh * NIB + ib
                    v.wait_ge(s_pe, rep * GG + (G0 if bh == 0 else GG))
                    if rep >= 1:
                        # scm tile reuse: previous rep's exp must be done
                        v.wait_ge(s_E, (rep - 1) * NSM + idx + 1)
                    v.scalar_tensor_tensor(
                        sc_t[idx], mask_t[idx], -1e30, sc_t[idx],
                        Alu.mult, Alu.add).then_inc(s_scm, 1)

            v.wait_ge(s_qka[0], 16)
            for rep in range(reps):
                for gg in range(GG):
                    bh_, size_, off_ = GLIST[gg]
                    if rep == 0 and bh_ == 0 and off_ < 16 <= off_ + size_:
                        v.wait_ge(s_qk0c, 16)
                    tbuild(v, rep, gg)
                    if gg == G0 + 2:
                        scm(v, rep, 0)  # bh0 softmax prep, overlapped
                scm(v, rep, 1)
                for idx in range(NSM):
                    v.wait_ge(s_E, rep * NSM + idx + 1)
                    if rep >= 1:
                        # W tile reuse: all of the previous rep's output DMAs
                        # on the owning queue must be done (conservative --
                        # cross-DMA order within a queue isn't assumed)
                        qs = s_wact if idx < NSM // 2 else s_wsp
                        v.wait_ge(qs, 16 * (NSM // 2) * rep)
                    v.reciprocal(r_t[idx], sums_t[idx])
                    v.drain()  # r is a scalar operand of the next op
                    v.tensor_scalar_mul(W_t[idx], E_t[idx],
                                        r_t[idx]).then_inc(s_W, 1)

        @block.scalar
        def _(a):
            for rep in range(reps):
                for gg in range(GG):
                    _, size, _ = GLIST[gg]
                    a.wait_ge(s_T, rep * GG + gg + 1)
                    T = T_t[(rep * GG + gg) % 3]
                    a.activation(T[:, 0:size * LQ], T[:, 0:size * LQ],
                                 AF.Silu).then_inc(s_S, 1)
                for idx in range(NSM):
                    a.wait_ge(s_scm, rep * NSM + idx + 1)
                    if rep >= 1:
                        # E/sums tile reuse: previous rep's W-scale must be done
                        a.wait_ge(s_W, (rep - 1) * NSM + idx + 1)
                    a.activation(E_t[idx], sc_t[idx], AF.Exp,
                                 accum_out=sums_t[idx]).then_inc(s_E, 1)
                # early output tiles on ACT's HWDGE queue (waits are
                # already satisfied by the time the last exp retires)
                for idx in range(NSM // 2):
                    bh, ib = divmod(idx, NIB)
                    a.wait_ge(s_W, rep * NSM + idx + 1)
                    a.dma_start(out=w_d[bh, ib * 128:(ib + 1) * 128, :],
                                in_=W_t[idx]).then_inc(s_wact, 16)


        @block.tensor
        def _(t):
            t.wait_ge(s_qk0b, 16)  # a2 rides in its own tiny chunk
            for rep in range(reps):
                for gg in range(GG):
                    bh, size, off = GLIST[gg]
                    if rep == 0 and bh == 1 and off == 0:
                        t.wait_ge(s_qka[1], 16)
                    if rep >= 1 and off == 0:
                        # sc bank reuse: previous rep's exp must have consumed it
                        t.wait_ge(s_E, (rep - 1) * NSM + NIB * (bh + 1))
                    t.wait_ge(s_S, rep * GG + gg + 1)
                    T = T_t[(rep * GG + gg) % 3]
                    for pl in range(size):
                        p = off + pl
                        for ib in range(NIB):
                            ins = nc.tensor.matmul(
                                sc_t[bh * NIB + ib][:, 2 * p:2 * p + 2],
                                T[:, pl * LQ + ib * 128: pl * LQ + (ib + 1) * 128],
                                a2(bh),
                                start=True, stop=True)
                    ins.then_inc(s_pe, 1)

    return nc


def _shard(q, k, a, mask):
    qf = q.reshape(B * H, LQ, D)
    kf = k.reshape(B * H, LK, D)
    mf = mask.reshape(B * H, LQ, LK)
    af = np.ascontiguousarray(
        np.broadcast_to(a.reshape(1, H, D), (B, H, D))).reshape(B * H, D)
    in_maps = []
    for c in range(NCORES):
        sl = slice(NBH * c, NBH * (c + 1))
        qT = qf[sl].transpose(0, 2, 1)                                # [NBH,64,LQ]
        kp = kf[sl].reshape(NBH, NPAIR, 2, D).transpose(0, 2, 3, 1)   # [NBH,2,D,NPAIR]
        qka = np.zeros((NBH, 128, QKA), np.float32)
        qka[:, 0:64, 0:LQ] = qT
        qka[:, 64:128, 0:LQ] = qT
        qka[:, :, LQ:LQ + NPAIR] = kp.reshape(NBH, 128, NPAIR)
        for j in range(NBH):
            qka[j, 0:64, LQ + NPAIR] = af[NBH * c + j]
            qka[j, 64:128, LQ + NPAIR + 1] = af[NBH * c + j]
        masku8 = np.ascontiguousarray(mf[sl]).astype(np.uint8)
        in_maps.append(dict(qka=qka, masku8=masku8))
    return in_maps


def kernel(q, k, attention, mask):
    global _built
    q = np.asarray(q, np.float32)
    k = np.asarray(k, np.float32)
    a = np.asarray(attention, np.float32)
    mask = np.asarray(mask).astype(bool)

    in_maps = _shard(q, k, a, mask)
    if _built is None:
        _built = _build()
    res = run_bass_kernel_spmd(_built, in_maps, core_ids=list(range(NCORES)))
    w = np.stack([res.results[c]["w"] for c in range(NCORES)], axis=0)
    return w.reshape(B, H, LQ, LK).astype(np.float32)



# revision 3
# speedup vs baseline: 3.4010x; 3.4010x over previous
"""GATv2 attention-weights kernel for 8 Trainium2 NeuronCores.

Problem (per full input):
    q: (2, 8, 384, 64) f32, k: (2, 8, 384, 64) f32,
    attention: (1, 8, 1, 1, 64) f32, mask: (2, 8, 384, 384) bool
    scores[b,h,i,j] = sum_d silu(q[b,h,i,d] + k[b,h,j,d]) * attention[h,d]
    out = softmax over j with mask (-inf before, 0 after)

Sharding: data-parallel over the 16 (b,h) pairs, 2 per core.

Algorithm (separable-score trick): silu(s) = s/2 + g(s) with g even;
g(s) ~= sum_n alpha_n cos(n*w0*s) (least-squares fit, w0 = pi/12, n<=10).
Each cosine term splits exactly:
    cos(n w0 (q+k)) = cos(n w0 q)cos(n w0 k) - sin(n w0 q)sin(n w0 k)
so the (i,j) score matrix becomes a plain matmul over (d, harmonic):
    score_ij = sum_d a_d k_jd/2                          (q-part is j-const,
                                                          softmax-invariant)
             + sum_n alpha_n sum_d a_d cos(n w0 (q+k))   (via PE)
Per-core device pipeline:
  - ACT: base features T1 = (cos(w0 x) | sin(w0 x)) packed on the partition
    axis as (cs, d) via one Sin activation with a per-partition bias column
    (pi/2 top half / 0 bottom); C1 = cos(w0 x) on all partitions.
    Arguments stay inside the Sin table's valid range [-pi, pi].
  - DVE: harmonic chain T_{n+1} = D1 . T_n - T_{n-1} (D1 = 2*C1) in fp16
    (2x DVE mode), k-side folded by +-a_d*alpha_n per-partition columns
    (fp16 4x mode); layout [(cs,d)=128 partitions, (bh,x)=768 free].
  - PE: one 128-wide fp16 matmul per (harmonic, bh, i-block) accumulating
    scores into 6 PSUM tiles, plus a rank-1 term for the linear part
    (ones x a_d k/4 over all 128 partitions).
  - Masked softmax as usual: (mask*-1e30)+scores on DVE, exp with fused
    row-sum on ACT, reciprocal + scale on DVE. Scores are bounded (|s|<8):
    exp cannot overflow, no row-max pass needed.
"""

import numpy as np
from contextlib import ExitStack

import concourse.bass as bass
from concourse import mybir
from concourse.bass_utils import run_bass_kernel_spmd

B, H, LQ, LK, D = 2, 8, 384, 384, 64
NCORES = 8
NBH = (B * H) // NCORES        # 2 (b,h) pairs per core
NIB = LQ // 128                # 3 i-blocks
NSM = NBH * NIB                # 6 softmax tiles
FW = NBH * LQ                  # 768 free width (bh-packed)

NH = 10                        # harmonics
W0 = 0.2617993877991494        # pi/12
ALPHA = [2.914250703, -2.50581741, -0.08424785112, -0.2465759164,
         -0.01425584389, -0.04962360035, -0.00106553002, -0.009994115834,
         -0.0004242872451, -0.0015243423, -0.0004320355077]
NCOL = 3 + 2 * NH + NBH        # consts columns

_f32 = mybir.dt.float32
_f16 = mybir.dt.float16
_u8 = mybir.dt.uint8

_built = None  # cache across calls


def _build(reps=1):
    # reps > 1 unrolls the whole computation N times inside one program
    # (used only for steady-state timing; the grading path uses reps=1).
    AF = mybir.ActivationFunctionType
    Alu = mybir.AluOpType

    nc = bass.Bass("TRN2", target_bir_lowering=False, debug=False,
                   num_devices=NCORES)

    qf_d = nc.dram_tensor("qf", [128, FW], _f32, kind="ExternalInput").ap()
    kf_d = nc.dram_tensor("kf", [128, FW], _f32, kind="ExternalInput").ap()
    cons_d = nc.dram_tensor("cons", [128, NCOL], _f32, kind="ExternalInput").ap()
    mask_d = nc.dram_tensor("masku8", [NBH, LQ, LK], _u8, kind="ExternalInput").ap()
    w_d = nc.dram_tensor("w", [NBH, LQ, LK], _f32, kind="ExternalOutput").ap()

    qf_t = nc.alloc_sbuf_tensor("qf_t", [128, FW], _f32).ap()
    kf_t = nc.alloc_sbuf_tensor("kf_t", [128, FW], _f32).ap()
    cons_t = nc.alloc_sbuf_tensor("cons_t", [128, NCOL], _f32).ap()
    mask_t = [nc.alloc_sbuf_tensor(f"mask_t{i}", [128, LK], _u8).ap()
              for i in range(NSM)]
    # feature tensors (fp16), harmonic index 1..NH
    Tq = [None] + [nc.alloc_sbuf_tensor(f"Tq{n}", [128, FW], _f16).ap()
                   for n in range(1, NH + 1)]
    Tk = [None] + [nc.alloc_sbuf_tensor(f"Tk{n}", [128, FW], _f16).ap()
                   for n in range(1, NH + 1)]
    Fk = [None] + [nc.alloc_sbuf_tensor(f"Fk{n}", [128, FW], _f16).ap()
                   for n in range(1, NH + 1)]
    D1q = nc.alloc_sbuf_tensor("D1q", [128, FW], _f16).ap()
    D1k = nc.alloc_sbuf_tensor("D1k", [128, FW], _f16).ap()
    lk_t = nc.alloc_sbuf_tensor("lk_t", [128, FW], _f16).ap()
    ones_t = nc.alloc_sbuf_tensor("ones_t", [128, 128], _f16).ap()
    E_t = [nc.alloc_sbuf_tensor(f"E{i}", [128, LK], _f32).ap()
           for i in range(NSM)]
    W_t = [nc.alloc_sbuf_tensor(f"W{i}", [128, LK], _f32).ap()
           for i in range(NSM)]
    sums_t = [nc.alloc_sbuf_tensor(f"sums{i}", [128, 1], _f32).ap()
              for i in range(NSM)]
    r_t = [nc.alloc_sbuf_tensor(f"r{i}", [128, 1], _f32).ap()
           for i in range(NSM)]
    sc_t = [nc.alloc_psum_tensor(f"sc{i}", [128, LK], _f32).ap()
            for i in range(NSM)]

    col = lambda c: cons_t[:, c:c + 1]
    BIAS_T1, BIAS_C1, T0COL = 0, 1, 2
    fold_c = lambda n, bh: 3 + 2 * (n - 1) + bh
    lin_c = lambda bh: 3 + 2 * NH + bh
    bsl = lambda bh: slice(bh * LQ, (bh + 1) * LQ)   # free-slice of one bh

    with ExitStack() as ctx:
        s_cons = ctx.enter_context(nc.semaphore("s_cons"))
        s_qf = ctx.enter_context(nc.semaphore("s_qf"))
        s_kf = ctx.enter_context(nc.semaphore("s_kf"))
        s_mask = ctx.enter_context(nc.semaphore("s_mask"))
        s_b = ctx.enter_context(nc.semaphore("s_b"))      # ACT bases, 4/rep
        s_d = ctx.enter_context(nc.semaphore("s_d"))      # D1s, 2/rep
        s_q = ctx.enter_context(nc.semaphore("s_q"))      # q chain, NH-1/rep
        s_f0 = ctx.enter_context(nc.semaphore("s_f0"))    # k folds bh0, NH/rep
        s_f1 = ctx.enter_context(nc.semaphore("s_f1"))    # k folds bh1, NH/rep
        s_lin = ctx.enter_context(nc.semaphore("s_lin"))  # lk+ones, 1/rep
        s_mm = ctx.enter_context(nc.semaphore("s_mm"))    # PE tile stops, 6/rep
        s_scm = ctx.enter_context(nc.semaphore("s_scm"))
        s_E = ctx.enter_context(nc.semaphore("s_E"))
        s_W = ctx.enter_context(nc.semaphore("s_W"))
        s_out = ctx.enter_context(nc.semaphore("s_out"))
        block = ctx.enter_context(nc.Block())

        @block.sync
        def _(sp):
            sp.dma_start(out=cons_t, in_=cons_d).then_inc(s_cons, 16)
            sp.dma_start(out=qf_t, in_=qf_d).then_inc(s_qf, 16)
            sp.dma_start(out=kf_t, in_=kf_d).then_inc(s_kf, 16)
            for idx in range(NSM):
                bh, ib = divmod(idx, NIB)
                sp.dma_start(out=mask_t[idx],
                             in_=mask_d[bh, ib * 128:(ib + 1) * 128, :]
                             ).then_inc(s_mask, 16)
            for rep in range(reps):
                for idx in range(NSM):
                    bh, ib = divmod(idx, NIB)
                    sp.wait_ge(s_W, rep * NSM + idx + 1)
                    sp.dma_start(out=w_d[bh, ib * 128:(ib + 1) * 128, :],
                                 in_=W_t[idx]).then_inc(s_out, 16)
            sp.wait_ge(s_out, 16 * NSM * reps)

        @block.scalar
        def _(a):
            a.wait_ge(s_cons, 16)
            a.wait_ge(s_qf, 16)
            a.wait_ge(s_kf, 16)
            for rep in range(reps):
                if rep >= 1:
                    # feature/base tensors reusable once all prior-rep
                    # matmuls retired
                    a.wait_ge(s_mm, NSM * rep)
                a.activation(Tq[1], qf_t, AF.Sin,
                             bias=col(BIAS_T1), scale=W0).then_inc(s_b, 1)
                a.activation(D1q, qf_t, AF.Sin,
                             bias=col(BIAS_C1), scale=W0).then_inc(s_b, 1)
                a.activation(Tk[1], kf_t, AF.Sin,
                             bias=col(BIAS_T1), scale=W0).then_inc(s_b, 1)
                a.activation(D1k, kf_t, AF.Sin,
                             bias=col(BIAS_C1), scale=W0).then_inc(s_b, 1)
                for idx in range(NSM):
                    a.wait_ge(s_scm, rep * NSM + idx + 1)
                    if rep >= 1:
                        a.wait_ge(s_W, (rep - 1) * NSM + idx + 1)
                    a.activation(E_t[idx], sc_t[idx], AF.Exp,
                                 accum_out=sums_t[idx]).then_inc(s_E, 1)

        @block.vector
        def _(v):
            for rep in range(reps):
                if rep >= 1:
                    v.wait_ge(s_mm, NSM * rep)
                # D1 = 2*cos(w0 x) (in place over the ACT cos output)
                v.wait_ge(s_b, 4 * rep + 2)
                v.tensor_scalar_mul(D1q, D1q, 2.0).then_inc(s_d, 1)
                v.wait_ge(s_b, 4 * rep + 4)
                v.tensor_scalar_mul(D1k, D1k, 2.0).then_inc(s_d, 1)
                # linear-term features: lk = a_d k/4 per bh; ones for lhsT
                if rep == 0:
                    v.memset(ones_t, 1.0)
                v.tensor_scalar_mul(lk_t[:, bsl(0)], kf_t[:, bsl(0)],
                                    col(lin_c(0)))
                v.tensor_scalar_mul(lk_t[:, bsl(1)], kf_t[:, bsl(1)],
                                    col(lin_c(1))).then_inc(s_lin, 1)
                v.tensor_scalar_mul(Fk[1][:, bsl(0)], Tk[1][:, bsl(0)],
                                    col(fold_c(1, 0))).then_inc(s_f0, 1)
                v.tensor_scalar_mul(Fk[1][:, bsl(1)], Tk[1][:, bsl(1)],
                                    col(fold_c(1, 1))).then_inc(s_f1, 1)
                # harmonic chains: q full-width, k per-bh (bh0 first so its
                # tiles retire early and the softmax tail overlaps bh1 work)
                for n in range(2, NH + 1):
                    # q side
                    v.tensor_tensor(Tq[n], D1q, Tq[n - 1], Alu.mult)
                    if n == 2:
                        ins = v.tensor_scalar(Tq[n], Tq[n], col(T0COL), None,
                                              Alu.subtract)
                    else:
                        ins = v.tensor_tensor(Tq[n], Tq[n], Tq[n - 2],
                                              Alu.subtract)
                    ins.then_inc(s_q, 1)
                    # k side, bh0 then bh1, fold right after
                    for bh, s_f in ((0, s_f0), (1, s_f1)):
                        sl = bsl(bh)
                        v.tensor_tensor(Tk[n][:, sl], D1k[:, sl],
                                        Tk[n - 1][:, sl], Alu.mult)
                        if n == 2:
                            v.tensor_scalar(Tk[n][:, sl], Tk[n][:, sl],
                                            col(T0COL), None, Alu.subtract)
                        else:
                            v.tensor_tensor(Tk[n][:, sl], Tk[n][:, sl],
                                            Tk[n - 2][:, sl], Alu.subtract)
                        v.tensor_scalar_mul(Fk[n][:, sl], Tk[n][:, sl],
                                            col(fold_c(n, bh))).then_inc(s_f, 1)
                # masked softmax prep + finish
                if rep == 0:
                    v.wait_ge(s_mask, 16 * NSM)
                for idx in range(NSM):
                    v.wait_ge(s_mm, rep * NSM + idx + 1)
                    if rep >= 1:
                        v.wait_ge(s_E, (rep - 1) * NSM + idx + 1)
                    v.scalar_tensor_tensor(
                        sc_t[idx], mask_t[idx], -1e30, sc_t[idx],
                        Alu.mult, Alu.add).then_inc(s_scm, 1)
                for idx in range(NSM):
                    v.wait_ge(s_E, rep * NSM + idx + 1)
                    if rep >= 1:
                        v.wait_ge(s_out, 16 * ((rep - 1) * NSM + idx + 1))
                    v.reciprocal(r_t[idx], sums_t[idx])
                    v.drain()  # r is a scalar operand of the next op
                    v.tensor_scalar_mul(W_t[idx], E_t[idx],
                                        r_t[idx]).then_inc(s_W, 1)

        @block.tensor
        def _(t):
            for rep in range(reps):
                # rank-1 linear term opens each tile's accumulation group
                t.wait_ge(s_lin, rep + 1)
                for idx in range(NSM):
                    bh, ib = divmod(idx, NIB)
                    if rep >= 1:
                        # PSUM bank reusable once prior rep's exp consumed it
                        t.wait_ge(s_E, (rep - 1) * NSM + idx + 1)
                    t.matmul(sc_t[idx], ones_t[:, 0:128], lk_t[:, bsl(bh)],
                             start=True, stop=False)
                for n in range(1, NH + 1):
                    if n >= 2:
                        t.wait_ge(s_q, rep * (NH - 1) + n - 1)
                    for bh, s_f in ((0, s_f0), (1, s_f1)):
                        t.wait_ge(s_f, rep * NH + n)
                        for ib in range(NIB):
                            idx = bh * NIB + ib
                            ins = t.matmul(
                                sc_t[idx],
                                Tq[n][:, bh * LQ + ib * 128:
                                      bh * LQ + (ib + 1) * 128],
                                Fk[n][:, bsl(bh)],
                                start=False, stop=(n == NH))
                            if n == NH:
                                ins.then_inc(s_mm, 1)

    return nc


def _shard(q, k, a, mask):
    qf = q.reshape(B * H, LQ, D)
    kf = k.reshape(B * H, LK, D)
    mf = mask.reshape(B * H, LQ, LK)
    af = np.ascontiguousarray(
        np.broadcast_to(a.reshape(1, H, D), (B, H, D))).reshape(B * H, D)
    in_maps = []
    for c in range(NCORES):
        sl = slice(NBH * c, NBH * (c + 1))
        # features layout: [partition (cs,d) = 128, free (bh, x) = 768],
        # x replicated across the cs halves
        qT = qf[sl].transpose(0, 2, 1)           # [NBH, 64, LQ]
        kT = kf[sl].transpose(0, 2, 1)
        qF = np.zeros((128, NBH, LQ), np.float32)
        kF = np.zeros((128, NBH, LQ), np.float32)
        for half in range(2):
            qF[half * 64:(half + 1) * 64] = qT.transpose(1, 0, 2)
            kF[half * 64:(half + 1) * 64] = kT.transpose(1, 0, 2)
        cons = np.zeros((128, NCOL), np.float32)
        cons[0:64, 0] = np.pi / 2                # T1 bias: cos top, sin bottom
        cons[:, 1] = np.pi / 2                   # C1 bias: cos everywhere
        cons[0:64, 2] = 1.0                      # T0 = (1 | 0)
        sign = np.concatenate([np.ones(64), -np.ones(64)]).astype(np.float32)
        for bh in range(NBH):
            ac = af[NBH * c + bh]
            ad = np.concatenate([ac, ac]).astype(np.float32)   # per (cs,d)
            for n in range(1, NH + 1):
                cons[:, 3 + 2 * (n - 1) + bh] = sign * ad * np.float32(ALPHA[n])
            cons[:, 3 + 2 * NH + bh] = ad / 4.0
        masku8 = np.ascontiguousarray(mf[sl]).astype(np.uint8)
        in_maps.append(dict(qf=qF.reshape(128, FW), kf=kF.reshape(128, FW),
                            cons=cons, masku8=masku8))
    return in_maps


def kernel(q, k, attention, mask):
    global _built
    q = np.asarray(q, np.float32)
    k = np.asarray(k, np.float32)
    a = np.asarray(attention, np.float32)
    mask = np.asarray(mask).astype(bool)

    in_maps = _shard(q, k, a, mask)
    if _built is None:
        _built = _build()
    res = run_bass_kernel_spmd(_built, in_maps, core_ids=list(range(NCORES)))
    w = np.stack([res.results[c]["w"] for c in range(NCORES)], axis=0)
    return w.reshape(B, H, LQ, LK).astype(np.float32)


# revision 7
# speedup vs baseline: 3.6810x; 1.0823x over previous
"""GATv2 attention-weights kernel for 8 Trainium2 NeuronCores.

Problem (per full input):
    q: (2, 8, 384, 64) f32, k: (2, 8, 384, 64) f32,
    attention: (1, 8, 1, 1, 64) f32, mask: (2, 8, 384, 384) bool
    scores[b,h,i,j] = sum_d silu(q[b,h,i,d] + k[b,h,j,d]) * attention[h,d]
    out = softmax over j with mask (-inf before, 0 after)

Sharding: data-parallel over the 16 (b,h) pairs, 2 per core.

Algorithm (separable-score trick): silu(s) = s/2 + g(s) with g even;
g(s) ~= sum_n alpha_n cos(n*w0*s) (least-squares fit, w0 = pi/12, n<=NH).
Each cosine term splits exactly:
    cos(n w0 (q+k)) = cos(n w0 q)cos(n w0 k) - sin(n w0 q)sin(n w0 k)
so the (i,j) score matrix becomes a plain matmul over (d, harmonic):
    score_ij = sum_d a_d k_jd/2                          (q-part is j-const,
                                                          softmax-invariant)
             + sum_n alpha_n sum_d a_d cos(n w0 (q+k))   (via PE)
Per-core device pipeline:
  - ACT: base features T1 = (cos(w0 x) | sin(w0 x)) packed on the partition
    axis as (cs, d) via one Sin activation with a per-partition bias column
    (pi/2 top half / 0 bottom); arguments stay inside the Sin table's valid
    range [-pi, pi].  Higher harmonics CANNOT use the Sin table (range), so:
  - DVE: harmonic chains T_{n+1} = D1 . T_n - T_{n-1} (D1 = 2*cos(w0 x)
    replicated over both cs halves) in fp16 (2x DVE mode).  DVE runs the
    q chain (768 wide, both bh) and the bh0 k chain; the bh1 k chain runs
    on the otherwise-idle Pool/GPSIMD engine.  k-features are folded by
    +-a_d*alpha_n per-partition columns (fp16 4x mode on DVE).
  - PE: one 128-wide fp16 matmul per (harmonic, bh, i-block) accumulating
    scores into 6 PSUM tiles, plus a rank-1 term for the linear part
    (ones x a_d k/4 over all 128 partitions).  bh0 features finish first
    so bh0 tiles retire early and the softmax tail overlaps bh1 work.
  - Masked softmax as usual: (mask*-1e30)+scores on DVE, exp with fused
    row-sum on ACT, reciprocal + scale on DVE. Scores are bounded (|s|<8):
    exp cannot overflow, no row-max pass needed.
"""

import numpy as np
from contextlib import ExitStack

import concourse.bass as bass
from concourse import mybir
from concourse.bass_utils import run_bass_kernel_spmd

B, H, LQ, LK, D = 2, 8, 384, 384, 64
NCORES = 8
NBH = (B * H) // NCORES        # 2 (b,h) pairs per core
NIB = LQ // 128                # 3 i-blocks
NSM = NBH * NIB                # 6 softmax tiles
FW = NBH * LQ                  # 768 free width (bh-packed)

NH = 10                        # harmonics
W0 = 0.2617993877991494        # pi/12
ALPHA = [2.914250703, -2.50581741, -0.08424785112, -0.2465759164,
         -0.01425584389, -0.04962360035, -0.00106553002, -0.009994115834,
         -0.0004242872451, -0.0015243423, -0.0004320355077]
NCOL = 3 + 2 * NH + NBH        # consts columns

_f32 = mybir.dt.float32
_f16 = mybir.dt.float16
_u8 = mybir.dt.uint8

_built = None  # cache across calls


def _build(reps=1):
    # reps > 1 unrolls the whole computation N times inside one program
    # (used only for steady-state timing; the grading path uses reps=1).
    AF = mybir.ActivationFunctionType
    Alu = mybir.AluOpType

    nc = bass.Bass("TRN2", target_bir_lowering=False, debug=False,
                   num_devices=NCORES)

    qf_d = nc.dram_tensor("qf", [128, FW], _f32, kind="ExternalInput").ap()
    kf_d = nc.dram_tensor("kf", [128, FW], _f32, kind="ExternalInput").ap()
    cons_d = nc.dram_tensor("cons", [128, NCOL], _f32, kind="ExternalInput").ap()
    mask_d = nc.dram_tensor("masku8", [NBH, LQ, LK], _u8, kind="ExternalInput").ap()
    w_d = nc.dram_tensor("w", [NBH, LQ, LK], _f32, kind="ExternalOutput").ap()

    qf_t = nc.alloc_sbuf_tensor("qf_t", [128, FW], _f32).ap()
    kf_t = nc.alloc_sbuf_tensor("kf_t", [128, FW], _f32).ap()
    cons_t = nc.alloc_sbuf_tensor("cons_t", [128, NCOL], _f32).ap()
    mask_t = [nc.alloc_sbuf_tensor(f"mask_t{i}", [128, LK], _u8).ap()
              for i in range(NSM)]
    # feature tensors (fp16), harmonic index 1..NH
    Tq = [None] + [nc.alloc_sbuf_tensor(f"Tq{n}", [128, FW], _f16).ap()
                   for n in range(1, NH + 1)]
    Tk = [None] + [nc.alloc_sbuf_tensor(f"Tk{n}", [128, FW], _f16).ap()
                   for n in range(1, NH + 1)]
    Fk = [None] + [nc.alloc_sbuf_tensor(f"Fk{n}", [128, FW], _f16).ap()
                   for n in range(1, NH + 1)]
    D1q = nc.alloc_sbuf_tensor("D1q", [128, FW], _f16).ap()
    D1k = nc.alloc_sbuf_tensor("D1k", [128, FW], _f16).ap()
    lk_t = nc.alloc_sbuf_tensor("lk_t", [128, FW], _f16).ap()
    ones_t = nc.alloc_sbuf_tensor("ones_t", [128, 128], _f16).ap()
    E_t = [nc.alloc_sbuf_tensor(f"E{i}", [128, LK], _f32).ap()
           for i in range(NSM)]
    W_t = [nc.alloc_sbuf_tensor(f"W{i}", [128, LK], _f32).ap()
           for i in range(NSM)]
    sums_t = [nc.alloc_sbuf_tensor(f"sums{i}", [128, 1], _f32).ap()
              for i in range(NSM)]
    r_t = [nc.alloc_sbuf_tensor(f"r{i}", [128, 1], _f32).ap()
           for i in range(NSM)]
    sc_t = [nc.alloc_psum_tensor(f"sc{i}", [128, LK], _f32).ap()
            for i in range(NSM)]

    col = lambda c: cons_t[:, c:c + 1]
    BIAS_T1, BIAS_C1, T0COL = 0, 1, 2
    fold_c = lambda n, bh: 3 + 2 * (n - 1) + bh
    lin_c = lambda bh: 3 + 2 * NH + bh
    bsl = lambda bh: slice(bh * LQ, (bh + 1) * LQ)   # free-slice of one bh

    with ExitStack() as ctx:
        s_cons = ctx.enter_context(nc.semaphore("s_cons"))
        s_qf = ctx.enter_context(nc.semaphore("s_qf"))
        s_kf = ctx.enter_context(nc.semaphore("s_kf"))
        s_mask = ctx.enter_context(nc.semaphore("s_mask"))
        s_b = ctx.enter_context(nc.semaphore("s_b"))      # ACT bases, 4/rep
        s_d = ctx.enter_context(nc.semaphore("s_d"))      # D1k then D1q, 2/rep
        s_q = ctx.enter_context(nc.semaphore("s_q"))      # q chain, NH-1/rep
        s_f0 = ctx.enter_context(nc.semaphore("s_f0"))    # k folds bh0, NH/rep
        s_f1 = ctx.enter_context(nc.semaphore("s_f1"))    # k folds bh1, NH/rep
        s_lin = ctx.enter_context(nc.semaphore("s_lin"))  # lk+ones, 1/rep
        s_mm = ctx.enter_context(nc.semaphore("s_mm"))    # PE tile stops, 6/rep
        s_scm = ctx.enter_context(nc.semaphore("s_scm"))
        s_E = ctx.enter_context(nc.semaphore("s_E"))
        s_W = ctx.enter_context(nc.semaphore("s_W"))
        s_out = ctx.enter_context(nc.semaphore("s_out"))
        block = ctx.enter_context(nc.Block())

        # k chain step on an engine `v` for one bh slice; returns last instr
        def kstep(v, n, sl):
            v.tensor_tensor(Tk[n][:, sl], D1k[:, sl], Tk[n - 1][:, sl],
                            Alu.mult)
            if n == 2:
                return v.tensor_scalar(Tk[n][:, sl], Tk[n][:, sl], col(T0COL),
                                       None, Alu.subtract)
            return v.tensor_tensor(Tk[n][:, sl], Tk[n][:, sl],
                                   Tk[n - 2][:, sl], Alu.subtract)

        def kfold(v, n, bh, s_f):
            sl = bsl(bh)
            v.tensor_scalar_mul(Fk[n][:, sl], Tk[n][:, sl],
                                col(fold_c(n, bh))).then_inc(s_f, 1)

        @block.sync
        def _(sp):
            sp.dma_start(out=cons_t, in_=cons_d).then_inc(s_cons, 16)
            sp.dma_start(out=kf_t, in_=kf_d).then_inc(s_kf, 16)
            for idx in range(NSM):
                bh, ib = divmod(idx, NIB)
                sp.dma_start(out=mask_t[idx],
                             in_=mask_d[bh, ib * 128:(ib + 1) * 128, :]
                             ).then_inc(s_mask, 16)
            for rep in range(reps):
                for idx in (1, 3, 5):
                    bh, ib = divmod(idx, NIB)
                    sp.wait_ge(s_W, rep * NSM + idx + 1)
                    sp.dma_start(out=w_d[bh, ib * 128:(ib + 1) * 128, :],
                                 in_=W_t[idx]).then_inc(s_out, 16)
            sp.wait_ge(s_out, 16 * NSM * reps)

        @block.scalar
        def _(a):
            a.dma_start(out=qf_t, in_=qf_d).then_inc(s_qf, 16)
            a.wait_ge(s_cons, 16)
            a.wait_ge(s_kf, 16)
            for rep in range(reps):
                if rep >= 1:
                    # feature/base tensors reusable once all prior-rep
                    # matmuls retired
                    a.wait_ge(s_mm, NSM * rep)
                a.activation(Tk[1], kf_t, AF.Sin,
                             bias=col(BIAS_T1), scale=W0).then_inc(s_b, 1)
                a.activation(D1k, kf_t, AF.Sin,
                             bias=col(BIAS_C1), scale=W0).then_inc(s_b, 1)
                if rep == 0:
                    a.wait_ge(s_qf, 16)
                a.activation(Tq[1], qf_t, AF.Sin,
                             bias=col(BIAS_T1), scale=W0).then_inc(s_b, 1)
                a.activation(D1q, qf_t, AF.Sin,
                             bias=col(BIAS_C1), scale=W0).then_inc(s_b, 1)
                for idx in range(NSM):
                    a.wait_ge(s_scm, rep * NSM + idx + 1)
                    if rep >= 1:
                        a.wait_ge(s_W, (rep - 1) * NSM + idx + 1)
                    a.activation(E_t[idx], sc_t[idx], AF.Exp,
                                 accum_out=sums_t[idx]).then_inc(s_E, 1)
                for idx in (0, 2, 4):
                    bh, ib = divmod(idx, NIB)
                    a.wait_ge(s_W, rep * NSM + idx + 1)
                    a.dma_start(out=w_d[bh, ib * 128:(ib + 1) * 128, :],
                                in_=W_t[idx]).then_inc(s_out, 16)

        @block.vector
        def _(v):
            for rep in range(reps):
                if rep >= 1:
                    v.wait_ge(s_mm, NSM * rep)
                if rep == 0:
                    v.wait_ge(s_kf, 16)
                    v.wait_ge(s_cons, 16)
                    v.memset(ones_t, 1.0)
                v.tensor_scalar_mul(lk_t[:, bsl(0)], kf_t[:, bsl(0)],
                                    col(lin_c(0)))
                v.tensor_scalar_mul(lk_t[:, bsl(1)], kf_t[:, bsl(1)],
                                    col(lin_c(1))).then_inc(s_lin, 1)
                # D1 = 2*cos(w0 x) (in place over the ACT cos output)
                v.wait_ge(s_b, 4 * rep + 2)
                v.tensor_scalar_mul(D1k, D1k, 2.0).then_inc(s_d, 1)
                kfold(v, 1, 0, s_f0)
                # prime the bh0 k chain while ACT finishes the q bases
                for n in (2, 3, 4):
                    kstep(v, n, bsl(0))
                    kfold(v, n, 0, s_f0)
                v.wait_ge(s_b, 4 * rep + 4)
                v.tensor_scalar_mul(D1q, D1q, 2.0).then_inc(s_d, 1)
                # interleave q chain with the rest of the bh0 k chain
                kq = 5
                for n in range(2, NH + 1):
                    v.tensor_tensor(Tq[n], D1q, Tq[n - 1], Alu.mult)
                    if n == 2:
                        ins = v.tensor_scalar(Tq[n], Tq[n], col(T0COL), None,
                                              Alu.subtract)
                    else:
                        ins = v.tensor_tensor(Tq[n], Tq[n], Tq[n - 2],
                                              Alu.subtract)
                    ins.then_inc(s_q, 1)
                    if kq <= NH:
                        kstep(v, kq, bsl(0))
                        kfold(v, kq, 0, s_f0)
                        kq += 1
                # masked softmax prep + finish
                if rep == 0:
                    v.wait_ge(s_mask, 16 * NSM)
                for idx in range(NSM):
                    v.wait_ge(s_mm, rep * NSM + idx + 1)
                    if rep >= 1:
                        v.wait_ge(s_E, (rep - 1) * NSM + idx + 1)
                    v.scalar_tensor_tensor(
                        sc_t[idx], mask_t[idx], -1e30, sc_t[idx],
                        Alu.mult, Alu.add).then_inc(s_scm, 1)
                for idx in range(NSM):
                    v.wait_ge(s_E, rep * NSM + idx + 1)
                    if rep >= 1 and idx == 0:
                        v.wait_ge(s_out, 16 * NSM * rep)
                    v.reciprocal(r_t[idx], sums_t[idx])
                    v.drain()  # r is a scalar operand of the next op
                    v.tensor_scalar_mul(W_t[idx], E_t[idx],
                                        r_t[idx]).then_inc(s_W, 1)

        @block.gpsimd
        def _(g):
            # bh1 k chain + folds on the otherwise-idle Pool engine
            for rep in range(reps):
                if rep >= 1:
                    g.wait_ge(s_mm, NSM * rep)
                g.wait_ge(s_b, 4 * rep + 1)
                kfold(g, 1, 1, s_f1)
                g.wait_ge(s_d, 2 * rep + 1)
                for n in range(2, NH + 1):
                    kstep(g, n, bsl(1))
                    kfold(g, n, 1, s_f1)

        @block.tensor
        def _(t):
            for rep in range(reps):
                # rank-1 linear term opens each tile's accumulation group
                t.wait_ge(s_lin, rep + 1)
                for idx in range(NSM):
                    bh, ib = divmod(idx, NIB)
                    if rep >= 1:
                        # PSUM bank reusable once prior rep's exp consumed it
                        t.wait_ge(s_E, (rep - 1) * NSM + idx + 1)
                    t.matmul(sc_t[idx], ones_t[:, 0:128], lk_t[:, bsl(bh)],
                             start=True, stop=False)
                for n in range(1, NH + 1):
                    if n == 1:
                        t.wait_ge(s_b, 4 * rep + 3)   # Tq[1] written by ACT
                    else:
                        t.wait_ge(s_q, rep * (NH - 1) + n - 1)
                    for bh, s_f in ((0, s_f0), (1, s_f1)):
                        t.wait_ge(s_f, rep * NH + n)
                        for ib in range(NIB):
                            idx = bh * NIB + ib
                            ins = t.matmul(
                                sc_t[idx],
                                Tq[n][:, bh * LQ + ib * 128:
                                      bh * LQ + (ib + 1) * 128],
                                Fk[n][:, bsl(bh)],
                                start=False, stop=(n == NH))
                            if n == NH:
                                ins.then_inc(s_mm, 1)

    return nc


def _shard(q, k, a, mask):
    qf = q.reshape(B * H, LQ, D)
    kf = k.reshape(B * H, LK, D)
    mf = mask.reshape(B * H, LQ, LK)
    af = np.ascontiguousarray(
        np.broadcast_to(a.reshape(1, H, D), (B, H, D))).reshape(B * H, D)
    in_maps = []
    for c in range(NCORES):
        sl = slice(NBH * c, NBH * (c + 1))
        # features layout: [partition (cs,d) = 128, free (bh, x) = 768],
        # x replicated across the cs halves
        qT = qf[sl].transpose(0, 2, 1)           # [NBH, 64, LQ]
        kT = kf[sl].transpose(0, 2, 1)
        qF = np.zeros((128, NBH, LQ), np.float32)
        kF = np.zeros((128, NBH, LQ), np.float32)
        for half in range(2):
            qF[half * 64:(half + 1) * 64] = qT.transpose(1, 0, 2)
            kF[half * 64:(half + 1) * 64] = kT.transpose(1, 0, 2)
        cons = np.zeros((128, NCOL), np.float32)
        cons[0:64, 0] = np.pi / 2                # T1 bias: cos top, sin bottom
        cons[:, 1] = np.pi / 2                   # C1 bias: cos everywhere
        cons[0:64, 2] = 1.0                      # T0 = (1 | 0)
        sign = np.concatenate([np.ones(64), -np.ones(64)]).astype(np.float32)
        for bh in range(NBH):
            ac = af[NBH * c + bh]
            ad = np.concatenate([ac, ac]).astype(np.float32)   # per (cs,d)
            for n in range(1, NH + 1):
                cons[:, 3 + 2 * (n - 1) + bh] = sign * ad * np.float32(ALPHA[n])
            cons[:, 3 + 2 * NH + bh] = ad / 4.0
        masku8 = np.ascontiguousarray(mf[sl]).astype(np.uint8)
        in_maps.append(dict(qf=qF.reshape(128, FW), kf=kF.reshape(128, FW),
                            cons=cons, masku8=masku8))
    return in_maps


def kernel(q, k, attention, mask):
    global _built
    q = np.asarray(q, np.float32)
    k = np.asarray(k, np.float32)
    a = np.asarray(attention, np.float32)
    mask = np.asarray(mask).astype(bool)

    in_maps = _shard(q, k, a, mask)
    if _built is None:
        _built = _build()
    res = run_bass_kernel_spmd(_built, in_maps, core_ids=list(range(NCORES)))
    w = np.stack([res.results[c]["w"] for c in range(NCORES)], axis=0)
    return w.reshape(B, H, LQ, LK).astype(np.float32)


# revision 8
# speedup vs baseline: 3.8538x; 1.0469x over previous
"""GATv2 attention-weights kernel for 8 Trainium2 NeuronCores.

Problem (per full input):
    q: (2, 8, 384, 64) f32, k: (2, 8, 384, 64) f32,
    attention: (1, 8, 1, 1, 64) f32, mask: (2, 8, 384, 384) bool
    scores[b,h,i,j] = sum_d silu(q[b,h,i,d] + k[b,h,j,d]) * attention[h,d]
    out = softmax over j with mask (-inf before, 0 after)

Sharding: data-parallel over the 16 (b,h) pairs, 2 per core.

Algorithm (separable-score trick): silu(s) = s/2 + g(s) with g even;
g(s) ~= sum_n alpha_n cos(n*w0*s) (least-squares fit, w0 = pi/12, n<=NH).
Each cosine term splits exactly:
    cos(n w0 (q+k)) = cos(n w0 q)cos(n w0 k) - sin(n w0 q)sin(n w0 k)
so the (i,j) score matrix becomes a plain matmul over (d, harmonic):
    score_ij = sum_d a_d k_jd/2                          (q-part is j-const,
                                                          softmax-invariant)
             + sum_n alpha_n sum_d a_d cos(n w0 (q+k))   (via PE)
Per-core device pipeline:
  - ACT: base features T1 = (cos(w0 x) | sin(w0 x)) packed on the partition
    axis as (cs, d) via one Sin activation with a per-partition bias column
    (pi/2 top half / 0 bottom); arguments stay inside the Sin table's valid
    range [-pi, pi].  Higher harmonics CANNOT use the Sin table (range), so:
  - DVE: harmonic chains T_{n+1} = D1 . T_n - T_{n-1} (D1 = 2*cos(w0 x)
    replicated over both cs halves) in fp16 (2x DVE mode).  DVE runs the
    q chain (768 wide, both bh) and the bh0 k chain; the bh1 k chain runs
    on the otherwise-idle Pool/GPSIMD engine.  k-features are folded by
    +-a_d*alpha_n per-partition columns (fp16 4x mode on DVE).
  - PE: one 128-wide fp16 matmul per (harmonic, bh, i-block) accumulating
    scores into 6 PSUM tiles, plus a rank-1 term for the linear part
    (ones x a_d k/4 over all 128 partitions).  bh0 features finish first
    so bh0 tiles retire early and the softmax tail overlaps bh1 work.
  - Masked softmax as usual: (mask*-1e30)+scores on DVE, exp with fused
    row-sum on ACT, reciprocal + scale on DVE. Scores are bounded (|s|<8):
    exp cannot overflow, no row-max pass needed.
"""

import numpy as np
from contextlib import ExitStack

import concourse.bass as bass
from concourse import mybir
from concourse.bass_utils import run_bass_kernel_spmd

B, H, LQ, LK, D = 2, 8, 384, 384, 64
NCORES = 8
NBH = (B * H) // NCORES        # 2 (b,h) pairs per core
NIB = LQ // 128                # 3 i-blocks
NSM = NBH * NIB                # 6 softmax tiles
FW = NBH * LQ                  # 768 free width (bh-packed)

NH = 10                        # harmonics
W0 = 0.2617993877991494        # pi/12
ALPHA = [2.914250703, -2.50581741, -0.08424785112, -0.2465759164,
         -0.01425584389, -0.04962360035, -0.00106553002, -0.009994115834,
         -0.0004242872451, -0.0015243423, -0.0004320355077]
NCOL = 3 + 2 * NH + NBH        # consts columns

_f32 = mybir.dt.float32
_f16 = mybir.dt.float16
_u8 = mybir.dt.uint8

_built = None  # cache across calls


def _build(reps=1):
    # reps > 1 unrolls the whole computation N times inside one program
    # (used only for steady-state timing; the grading path uses reps=1).
    AF = mybir.ActivationFunctionType
    Alu = mybir.AluOpType

    nc = bass.Bass("TRN2", target_bir_lowering=False, debug=False,
                   num_devices=NCORES)

    qf_d = nc.dram_tensor("qf", [128, FW], _f32, kind="ExternalInput").ap()
    kf_d = nc.dram_tensor("kf", [128, FW], _f32, kind="ExternalInput").ap()
    cons_d = nc.dram_tensor("cons", [128, NCOL], _f32, kind="ExternalInput").ap()
    mask_d = nc.dram_tensor("masku8", [NBH, LQ, LK], _u8, kind="ExternalInput").ap()
    w_d = nc.dram_tensor("w", [NBH, LQ, LK], _f32, kind="ExternalOutput").ap()

    qf_t = nc.alloc_sbuf_tensor("qf_t", [128, FW], _f32).ap()
    kf_t = nc.alloc_sbuf_tensor("kf_t", [128, FW], _f32).ap()
    cons_t = nc.alloc_sbuf_tensor("cons_t", [128, NCOL], _f32).ap()
    mask_t = [nc.alloc_sbuf_tensor(f"mask_t{i}", [128, LK], _u8).ap()
              for i in range(NSM)]
    # feature tensors (fp16), harmonic index 1..NH
    Tq = [None] + [nc.alloc_sbuf_tensor(f"Tq{n}", [128, FW], _f16).ap()
                   for n in range(1, NH + 1)]
    Tk = [None] + [nc.alloc_sbuf_tensor(f"Tk{n}", [128, FW], _f16).ap()
                   for n in range(1, NH + 1)]
    Fk = [None] + [nc.alloc_sbuf_tensor(f"Fk{n}", [128, FW], _f16).ap()
                   for n in range(1, NH + 1)]
    D1q = nc.alloc_sbuf_tensor("D1q", [128, FW], _f16).ap()
    D1k = nc.alloc_sbuf_tensor("D1k", [128, FW], _f16).ap()
    lk_t = nc.alloc_sbuf_tensor("lk_t", [128, FW], _f16).ap()
    ones_t = nc.alloc_sbuf_tensor("ones_t", [128, 128], _f16).ap()
    E_t = [nc.alloc_sbuf_tensor(f"E{i}", [128, LK], _f32).ap()
           for i in range(NSM)]
    W_t = [nc.alloc_sbuf_tensor(f"W{i}", [128, LK], _f32).ap()
           for i in range(NSM)]
    sums_t = [nc.alloc_sbuf_tensor(f"sums{i}", [128, 1], _f32).ap()
              for i in range(NSM)]
    r_t = [nc.alloc_sbuf_tensor(f"r{i}", [128, 1], _f32).ap()
           for i in range(NSM)]
    sc_t = [nc.alloc_psum_tensor(f"sc{i}", [128, LK], _f32).ap()
            for i in range(NSM)]

    col = lambda c: cons_t[:, c:c + 1]
    BIAS_T1, BIAS_C1, T0COL = 0, 1, 2
    fold_c = lambda n, bh: 3 + 2 * (n - 1) + bh
    lin_c = lambda bh: 3 + 2 * NH + bh
    bsl = lambda bh: slice(bh * LQ, (bh + 1) * LQ)   # free-slice of one bh

    with ExitStack() as ctx:
        s_cons = ctx.enter_context(nc.semaphore("s_cons"))
        s_qf = ctx.enter_context(nc.semaphore("s_qf"))
        s_kf = ctx.enter_context(nc.semaphore("s_kf"))
        s_mask = ctx.enter_context(nc.semaphore("s_mask"))
        s_b = ctx.enter_context(nc.semaphore("s_b"))      # ACT bases, 4/rep
        s_d = ctx.enter_context(nc.semaphore("s_d"))      # D1k then D1q, 2/rep
        s_q = ctx.enter_context(nc.semaphore("s_q"))      # q chain, NH-1/rep
        s_k1 = ctx.enter_context(nc.semaphore("s_k1"))    # Pool k1 steps, NH-1/rep
        s_f0 = ctx.enter_context(nc.semaphore("s_f0"))    # k folds bh0, NH/rep
        s_f1 = ctx.enter_context(nc.semaphore("s_f1"))    # k folds bh1, NH/rep
        s_lin = ctx.enter_context(nc.semaphore("s_lin"))  # lk+ones, 1/rep
        s_mm = ctx.enter_context(nc.semaphore("s_mm"))    # PE tile stops, 6/rep
        s_scm = ctx.enter_context(nc.semaphore("s_scm"))
        s_E = ctx.enter_context(nc.semaphore("s_E"))
        s_W = ctx.enter_context(nc.semaphore("s_W"))
        s_out = ctx.enter_context(nc.semaphore("s_out"))
        block = ctx.enter_context(nc.Block())

        # k chain step on an engine `v` for one bh slice; returns last instr
        def kstep(v, n, sl):
            v.tensor_tensor(Tk[n][:, sl], D1k[:, sl], Tk[n - 1][:, sl],
                            Alu.mult)
            if n == 2:
                return v.tensor_scalar(Tk[n][:, sl], Tk[n][:, sl], col(T0COL),
                                       None, Alu.subtract)
            return v.tensor_tensor(Tk[n][:, sl], Tk[n][:, sl],
                                   Tk[n - 2][:, sl], Alu.subtract)

        def kfold(v, n, bh, s_f):
            sl = bsl(bh)
            v.tensor_scalar_mul(Fk[n][:, sl], Tk[n][:, sl],
                                col(fold_c(n, bh))).then_inc(s_f, 1)

        @block.sync
        def _(sp):
            sp.dma_start(out=kf_t, in_=kf_d).then_inc(s_kf, 16)
            sp.dma_start(out=cons_t, in_=cons_d).then_inc(s_cons, 16)
            for idx in range(NSM):
                bh, ib = divmod(idx, NIB)
                sp.dma_start(out=mask_t[idx],
                             in_=mask_d[bh, ib * 128:(ib + 1) * 128, :]
                             ).then_inc(s_mask, 16)
            for rep in range(reps):
                for idx in (1, 3, 5):
                    bh, ib = divmod(idx, NIB)
                    sp.wait_ge(s_W, rep * NSM + idx + 1)
                    sp.dma_start(out=w_d[bh, ib * 128:(ib + 1) * 128, :],
                                 in_=W_t[idx]).then_inc(s_out, 16)
            sp.wait_ge(s_out, 16 * NSM * reps)

        @block.scalar
        def _(a):
            a.dma_start(out=qf_t, in_=qf_d).then_inc(s_qf, 16)
            a.wait_ge(s_cons, 16)
            a.wait_ge(s_kf, 16)
            for rep in range(reps):
                if rep >= 1:
                    # feature/base tensors reusable once all prior-rep
                    # matmuls retired
                    a.wait_ge(s_mm, NSM * rep)
                a.activation(Tk[1], kf_t, AF.Sin,
                             bias=col(BIAS_T1), scale=W0).then_inc(s_b, 1)
                a.activation(D1k, kf_t, AF.Sin,
                             bias=col(BIAS_C1), scale=W0).then_inc(s_b, 1)
                if rep == 0:
                    a.wait_ge(s_qf, 16)
                a.activation(Tq[1], qf_t, AF.Sin,
                             bias=col(BIAS_T1), scale=W0).then_inc(s_b, 1)
                a.activation(D1q, qf_t, AF.Sin,
                             bias=col(BIAS_C1), scale=W0).then_inc(s_b, 1)
                def f1fold(n):
                    if n >= 2:
                        a.wait_ge(s_k1, rep * (NH - 1) + n - 1)
                    sl = bsl(1)
                    a.activation(Fk[n][:, sl], Tk[n][:, sl], AF.Copy,
                                 scale=col(fold_c(n, 1))).then_inc(s_f1, 1)

                def expt(idx):
                    a.wait_ge(s_scm, rep * NSM + idx + 1)
                    if rep >= 1:
                        a.wait_ge(s_W, (rep - 1) * NSM + idx + 1)
                    a.activation(E_t[idx], sc_t[idx], AF.Exp,
                                 accum_out=sums_t[idx]).then_inc(s_E, 1)

                for n in range(1, NH - 1):
                    f1fold(n)
                for idx in range(NIB):          # bh0 tiles overlap the last
                    expt(idx)                   # Pool chain steps
                f1fold(NH - 1)
                f1fold(NH)
                for idx in range(NIB, NSM):
                    expt(idx)
                for idx in (0, 2, 4):
                    bh, ib = divmod(idx, NIB)
                    a.wait_ge(s_W, rep * NSM + idx + 1)
                    a.dma_start(out=w_d[bh, ib * 128:(ib + 1) * 128, :],
                                in_=W_t[idx]).then_inc(s_out, 16)

        @block.vector
        def _(v):
            for rep in range(reps):
                if rep >= 1:
                    v.wait_ge(s_mm, NSM * rep)
                if rep == 0:
                    v.wait_ge(s_kf, 16)
                    v.wait_ge(s_cons, 16)
                    v.memset(ones_t, 1.0)
                v.tensor_scalar_mul(lk_t[:, bsl(0)], kf_t[:, bsl(0)],
                                    col(lin_c(0)))
                v.tensor_scalar_mul(lk_t[:, bsl(1)], kf_t[:, bsl(1)],
                                    col(lin_c(1))).then_inc(s_lin, 1)
                # D1 = 2*cos(w0 x) (in place over the ACT cos output)
                v.wait_ge(s_b, 4 * rep + 2)
                v.tensor_scalar_mul(D1k, D1k, 2.0).then_inc(s_d, 1)
                kfold(v, 1, 0, s_f0)
                # prime the bh0 k chain while ACT finishes the q bases
                for n in (2, 3, 4):
                    kstep(v, n, bsl(0))
                    kfold(v, n, 0, s_f0)
                v.wait_ge(s_b, 4 * rep + 4)
                v.tensor_scalar_mul(D1q, D1q, 2.0).then_inc(s_d, 1)
                # interleave q chain with the rest of the bh0 k chain
                kq = 5
                for n in range(2, NH + 1):
                    v.tensor_tensor(Tq[n], D1q, Tq[n - 1], Alu.mult)
                    if n == 2:
                        ins = v.tensor_scalar(Tq[n], Tq[n], col(T0COL), None,
                                              Alu.subtract)
                    else:
                        ins = v.tensor_tensor(Tq[n], Tq[n], Tq[n - 2],
                                              Alu.subtract)
                    ins.then_inc(s_q, 1)
                    if kq <= NH:
                        kstep(v, kq, bsl(0))
                        kfold(v, kq, 0, s_f0)
                        kq += 1
                # masked softmax prep + finish
                if rep == 0:
                    v.wait_ge(s_mask, 16 * NSM)
                for idx in range(NSM):
                    v.wait_ge(s_mm, rep * NSM + idx + 1)
                    if rep >= 1:
                        v.wait_ge(s_E, (rep - 1) * NSM + idx + 1)
                    v.scalar_tensor_tensor(
                        sc_t[idx], mask_t[idx], -1e30, sc_t[idx],
                        Alu.mult, Alu.add).then_inc(s_scm, 1)
                for idx in range(NSM):
                    v.wait_ge(s_E, rep * NSM + idx + 1)
                    if rep >= 1 and idx == 0:
                        v.wait_ge(s_out, 16 * NSM * rep)
                    v.reciprocal(r_t[idx], sums_t[idx])
                    v.drain()  # r is a scalar operand of the next op
                    v.tensor_scalar_mul(W_t[idx], E_t[idx],
                                        r_t[idx]).then_inc(s_W, 1)

        @block.gpsimd
        def _(g):
            # bh1 k chain on the otherwise-idle Pool engine (folds on ACT)
            for rep in range(reps):
                if rep >= 1:
                    g.wait_ge(s_mm, NSM * rep)
                g.wait_ge(s_b, 4 * rep + 1)
                g.wait_ge(s_d, 2 * rep + 1)
                for n in range(2, NH + 1):
                    kstep(g, n, bsl(1)).then_inc(s_k1, 1)

        @block.tensor
        def _(t):
            for rep in range(reps):
                # two passes: all bh0 matmuls first (gated by DVE, fast),
                # then bh1 (gated by Pool/ACT) -- bh0 tiles retire early so
                # the softmax tail overlaps bh1 feature production
                t.wait_ge(s_lin, rep + 1)
                for bh, s_f in ((0, s_f0), (1, s_f1)):
                    for ib in range(NIB):
                        idx = bh * NIB + ib
                        if rep >= 1:
                            # PSUM bank reusable once prior exp consumed it
                            t.wait_ge(s_E, (rep - 1) * NSM + idx + 1)
                        t.matmul(sc_t[idx], ones_t[:, 0:128],
                                 lk_t[:, bsl(bh)], start=True, stop=False)
                    for n in range(1, NH + 1):
                        if bh == 0:
                            if n == 1:
                                t.wait_ge(s_b, 4 * rep + 3)  # Tq[1] by ACT
                            else:
                                t.wait_ge(s_q, rep * (NH - 1) + n - 1)
                        t.wait_ge(s_f, rep * NH + n)
                        for ib in range(NIB):
                            idx = bh * NIB + ib
                            ins = t.matmul(
                                sc_t[idx],
                                Tq[n][:, bh * LQ + ib * 128:
                                      bh * LQ + (ib + 1) * 128],
                                Fk[n][:, bsl(bh)],
                                start=False, stop=(n == NH))
                            if n == NH:
                                ins.then_inc(s_mm, 1)

    return nc


def _shard(q, k, a, mask):
    qf = q.reshape(B * H, LQ, D)
    kf = k.reshape(B * H, LK, D)
    mf = mask.reshape(B * H, LQ, LK)
    af = np.ascontiguousarray(
        np.broadcast_to(a.reshape(1, H, D), (B, H, D))).reshape(B * H, D)
    in_maps = []
    for c in range(NCORES):
        sl = slice(NBH * c, NBH * (c + 1))
        # features layout: [partition (cs,d) = 128, free (bh, x) = 768],
        # x replicated across the cs halves
        qT = qf[sl].transpose(0, 2, 1)           # [NBH, 64, LQ]
        kT = kf[sl].transpose(0, 2, 1)
        qF = np.zeros((128, NBH, LQ), np.float32)
        kF = np.zeros((128, NBH, LQ), np.float32)
        for half in range(2):
            qF[half * 64:(half + 1) * 64] = qT.transpose(1, 0, 2)
            kF[half * 64:(half + 1) * 64] = kT.transpose(1, 0, 2)
        cons = np.zeros((128, NCOL), np.float32)
        cons[0:64, 0] = np.pi / 2                # T1 bias: cos top, sin bottom
        cons[:, 1] = np.pi / 2                   # C1 bias: cos everywhere
        cons[0:64, 2] = 1.0                      # T0 = (1 | 0)
        sign = np.concatenate([np.ones(64), -np.ones(64)]).astype(np.float32)
        for bh in range(NBH):
            ac = af[NBH * c + bh]
            ad = np.concatenate([ac, ac]).astype(np.float32)   # per (cs,d)
            for n in range(1, NH + 1):
                cons[:, 3 + 2 * (n - 1) + bh] = sign * ad * np.float32(ALPHA[n])
            cons[:, 3 + 2 * NH + bh] = ad / 4.0
        masku8 = np.ascontiguousarray(mf[sl]).astype(np.uint8)
        in_maps.append(dict(qf=qF.reshape(128, FW), kf=kF.reshape(128, FW),
                            cons=cons, masku8=masku8))
    return in_maps


def kernel(q, k, attention, mask):
    global _built
    q = np.asarray(q, np.float32)
    k = np.asarray(k, np.float32)
    a = np.asarray(attention, np.float32)
    mask = np.asarray(mask).astype(bool)

    in_maps = _shard(q, k, a, mask)
    if _built is None:
        _built = _build()
    res = run_bass_kernel_spmd(_built, in_maps, core_ids=list(range(NCORES)))
    w = np.stack([res.results[c]["w"] for c in range(NCORES)], axis=0)
    return w.reshape(B, H, LQ, LK).astype(np.float32)


# revision 9
# speedup vs baseline: 4.1697x; 1.0820x over previous
"""GATv2 attention-weights kernel for 8 Trainium2 NeuronCores.

Problem (per full input):
    q: (2, 8, 384, 64) f32, k: (2, 8, 384, 64) f32,
    attention: (1, 8, 1, 1, 64) f32, mask: (2, 8, 384, 384) bool
    scores[b,h,i,j] = sum_d silu(q[b,h,i,d] + k[b,h,j,d]) * attention[h,d]
    out = softmax over j with mask (-inf before, 0 after)

Sharding: data-parallel over the 16 (b,h) pairs, 2 per core.

Algorithm (separable-score trick): silu(s) = s/2 + g(s) with g even;
g(s) ~= sum_n alpha_n cos(n*w0*s) (least-squares fit, w0 = pi/12, n<=NH).
Each cosine term splits exactly:
    cos(n w0 (q+k)) = cos(n w0 q)cos(n w0 k) - sin(n w0 q)sin(n w0 k)
so the (i,j) score matrix becomes a plain matmul over (d, harmonic):
    score_ij = sum_d a_d k_jd/2                          (q-part is j-const,
                                                          softmax-invariant)
             + sum_n alpha_n sum_d a_d cos(n w0 (q+k))   (via PE)
Per-core device pipeline:
  - ACT: base features T1 = (cos(w0 x) | sin(w0 x)) packed on the partition
    axis as (cs, d) via one Sin activation with a per-partition bias column
    (pi/2 top half / 0 bottom); arguments stay inside the Sin table's valid
    range [-pi, pi].  Higher harmonics CANNOT use the Sin table (range), so:
  - DVE: harmonic chains T_{n+1} = D1 . T_n - T_{n-1} (D1 = 2*cos(w0 x)
    replicated over both cs halves) in fp16 (2x DVE mode).  DVE runs the
    q chain (768 wide, both bh) and the bh0 k chain; the bh1 k chain runs
    on the otherwise-idle Pool/GPSIMD engine.  k-features are folded by
    +-a_d*alpha_n per-partition columns (fp16 4x mode on DVE).
  - PE: one 128-wide fp16 matmul per (harmonic, bh, i-block) accumulating
    scores into 6 PSUM tiles, plus a rank-1 term for the linear part
    (ones x a_d k/4 over all 128 partitions).  bh0 features finish first
    so bh0 tiles retire early and the softmax tail overlaps bh1 work.
  - Masked softmax as usual: (mask*-1e30)+scores on DVE, exp with fused
    row-sum on ACT, reciprocal + scale on DVE. Scores are bounded (|s|<8):
    exp cannot overflow, no row-max pass needed.
"""

import numpy as np
from contextlib import ExitStack

import concourse.bass as bass
from concourse import mybir
from concourse.bass_utils import run_bass_kernel_spmd

B, H, LQ, LK, D = 2, 8, 384, 384, 64
NCORES = 8
NBH = (B * H) // NCORES        # 2 (b,h) pairs per core
NIB = LQ // 128                # 3 i-blocks
NSM = NBH * NIB                # 6 softmax tiles
FW = NBH * LQ                  # 768 free width (bh-packed)

NH = 10                        # harmonics
W0 = 0.2617993877991494        # pi/12
ALPHA = [2.914250703, -2.50581741, -0.08424785112, -0.2465759164,
         -0.01425584389, -0.04962360035, -0.00106553002, -0.009994115834,
         -0.0004242872451, -0.0015243423, -0.0004320355077]
NCOL = 3 + 2 * NH + NBH        # consts columns

_f32 = mybir.dt.float32
_f16 = mybir.dt.float16
_u8 = mybir.dt.uint8

_built = None  # cache across calls


def _build(reps=1):
    # reps > 1 unrolls the whole computation N times inside one program
    # (used only for steady-state timing; the grading path uses reps=1).
    AF = mybir.ActivationFunctionType
    Alu = mybir.AluOpType

    nc = bass.Bass("TRN2", target_bir_lowering=False, debug=False,
                   num_devices=NCORES)

    qf_d = nc.dram_tensor("qf", [128, FW], _f32, kind="ExternalInput").ap()
    kf_d = nc.dram_tensor("kf", [128, FW], _f32, kind="ExternalInput").ap()
    cons_d = nc.dram_tensor("cons", [128, NCOL], _f32, kind="ExternalInput").ap()
    mask_d = nc.dram_tensor("masku8", [NBH, LQ, LK], _u8, kind="ExternalInput").ap()
    w_d = nc.dram_tensor("w", [NBH, LQ, LK], _f32, kind="ExternalOutput").ap()

    qf_t = nc.alloc_sbuf_tensor("qf_t", [128, FW], _f32).ap()
    kf_t = nc.alloc_sbuf_tensor("kf_t", [128, FW], _f32).ap()
    cons_t = nc.alloc_sbuf_tensor("cons_t", [128, NCOL], _f32).ap()
    mask_t = [nc.alloc_sbuf_tensor(f"mask_t{i}", [128, LK], _u8).ap()
              for i in range(NSM)]
    # feature tensors (fp16), harmonic index 1..NH
    Tq = [None] + [nc.alloc_sbuf_tensor(f"Tq{n}", [128, FW], _f16).ap()
                   for n in range(1, NH + 1)]
    Tk = [None] + [nc.alloc_sbuf_tensor(f"Tk{n}", [128, FW], _f16).ap()
                   for n in range(1, NH + 1)]
    Fk = [None] + [nc.alloc_sbuf_tensor(f"Fk{n}", [128, FW], _f16).ap()
                   for n in range(1, NH + 1)]
    D1q = nc.alloc_sbuf_tensor("D1q", [128, FW], _f16).ap()
    D1k = nc.alloc_sbuf_tensor("D1k", [128, FW], _f16).ap()
    lk_t = nc.alloc_sbuf_tensor("lk_t", [128, FW], _f16).ap()
    ones_t = nc.alloc_sbuf_tensor("ones_t", [128, 128], _f16).ap()
    E_t = [nc.alloc_sbuf_tensor(f"E{i}", [128, LK], _f32).ap()
           for i in range(NSM)]
    W_t = [nc.alloc_sbuf_tensor(f"W{i}", [128, LK], _f32).ap()
           for i in range(NSM)]
    sums_t = [nc.alloc_sbuf_tensor(f"sums{i}", [128, 1], _f32).ap()
              for i in range(NSM)]
    r_t = [nc.alloc_sbuf_tensor(f"r{i}", [128, 1], _f32).ap()
           for i in range(NSM)]
    sc_t = [nc.alloc_psum_tensor(f"sc{i}", [128, LK], _f32).ap()
            for i in range(NSM)]

    col = lambda c: cons_t[:, c:c + 1]
    BIAS_T1, BIAS_C1, T0COL = 0, 1, 2
    fold_c = lambda n, bh: 3 + 2 * (n - 1) + bh
    lin_c = lambda bh: 3 + 2 * NH + bh
    bsl = lambda bh: slice(bh * LQ, (bh + 1) * LQ)   # free-slice of one bh

    with ExitStack() as ctx:
        s_cons = ctx.enter_context(nc.semaphore("s_cons"))
        s_qf = ctx.enter_context(nc.semaphore("s_qf"))
        s_kf = ctx.enter_context(nc.semaphore("s_kf"))
        s_mask = ctx.enter_context(nc.semaphore("s_mask"))
        s_b = ctx.enter_context(nc.semaphore("s_b"))      # ACT bases, 4/rep
        s_d = ctx.enter_context(nc.semaphore("s_d"))      # D1k then D1q, 2/rep
        s_q = ctx.enter_context(nc.semaphore("s_q"))      # q chain, NH-1/rep
        s_k1 = ctx.enter_context(nc.semaphore("s_k1"))    # Pool k1 steps, NH-1/rep
        s_f0 = ctx.enter_context(nc.semaphore("s_f0"))    # k folds bh0, NH/rep
        s_f1 = ctx.enter_context(nc.semaphore("s_f1"))    # k folds bh1, NH/rep
        s_lin = ctx.enter_context(nc.semaphore("s_lin"))  # lk+ones, 1/rep
        s_mm = ctx.enter_context(nc.semaphore("s_mm"))    # PE tile stops, 6/rep
        s_scm = ctx.enter_context(nc.semaphore("s_scm"))
        s_E = ctx.enter_context(nc.semaphore("s_E"))
        s_W = ctx.enter_context(nc.semaphore("s_W"))
        s_out = ctx.enter_context(nc.semaphore("s_out"))
        block = ctx.enter_context(nc.Block())

        # k chain step on an engine `v` for one bh slice; returns last instr
        def kstep(v, n, sl):
            v.tensor_tensor(Tk[n][:, sl], D1k[:, sl], Tk[n - 1][:, sl],
                            Alu.mult)
            if n == 2:
                return v.tensor_scalar(Tk[n][:, sl], Tk[n][:, sl], col(T0COL),
                                       None, Alu.subtract)
            return v.tensor_tensor(Tk[n][:, sl], Tk[n][:, sl],
                                   Tk[n - 2][:, sl], Alu.subtract)

        def kfold(v, n, bh, s_f):
            sl = bsl(bh)
            v.tensor_scalar_mul(Fk[n][:, sl], Tk[n][:, sl],
                                col(fold_c(n, bh))).then_inc(s_f, 1)

        @block.sync
        def _(sp):
            sp.dma_start(out=kf_t, in_=kf_d).then_inc(s_kf, 16)
            for idx in range(NSM):
                bh, ib = divmod(idx, NIB)
                sp.dma_start(out=mask_t[idx],
                             in_=mask_d[bh, ib * 128:(ib + 1) * 128, :]
                             ).then_inc(s_mask, 16)
            for rep in range(reps):
                for idx in (1, 3, 5):
                    bh, ib = divmod(idx, NIB)
                    sp.wait_ge(s_W, rep * NSM + idx + 1)
                    sp.dma_start(out=w_d[bh, ib * 128:(ib + 1) * 128, :],
                                 in_=W_t[idx]).then_inc(s_out, 16)
            sp.wait_ge(s_out, 16 * NSM * reps)

        @block.scalar
        def _(a):
            a.dma_start(out=qf_t, in_=qf_d).then_inc(s_qf, 16)
            a.dma_start(out=cons_t, in_=cons_d).then_inc(s_cons, 16)
            a.wait_ge(s_cons, 16)
            a.wait_ge(s_kf, 16)
            for rep in range(reps):
                if rep >= 1:
                    # feature/base tensors reusable once all prior-rep
                    # matmuls retired
                    a.wait_ge(s_mm, NSM * rep)
                a.activation(Tk[1], kf_t, AF.Sin,
                             bias=col(BIAS_T1), scale=W0).then_inc(s_b, 1)
                a.activation(D1k, kf_t, AF.Sin,
                             bias=col(BIAS_C1), scale=W0).then_inc(s_b, 1)
                if rep == 0:
                    a.wait_ge(s_qf, 16)
                a.activation(Tq[1], qf_t, AF.Sin,
                             bias=col(BIAS_T1), scale=W0).then_inc(s_b, 1)
                a.activation(D1q, qf_t, AF.Sin,
                             bias=col(BIAS_C1), scale=W0).then_inc(s_b, 1)
                def f1fold(n):
                    if n >= 2:
                        a.wait_ge(s_k1, rep * (NH - 1) + n - 1)
                    sl = bsl(1)
                    a.activation(Fk[n][:, sl], Tk[n][:, sl], AF.Copy,
                                 scale=col(fold_c(n, 1))).then_inc(s_f1, 1)

                def expt(idx):
                    a.wait_ge(s_scm, rep * NSM + idx + 1)
                    if rep >= 1:
                        a.wait_ge(s_W, (rep - 1) * NSM + idx + 1)
                    a.activation(E_t[idx], sc_t[idx], AF.Exp,
                                 accum_out=sums_t[idx]).then_inc(s_E, 1)

                for n in range(1, NH - 1):
                    f1fold(n)
                for idx in range(NSM):
                    expt(idx)
                for idx in (0, 2, 4):
                    bh, ib = divmod(idx, NIB)
                    a.wait_ge(s_W, rep * NSM + idx + 1)
                    a.dma_start(out=w_d[bh, ib * 128:(ib + 1) * 128, :],
                                in_=W_t[idx]).then_inc(s_out, 16)

        @block.vector
        def _(v):
            for rep in range(reps):
                if rep >= 1:
                    v.wait_ge(s_mm, NSM * rep)
                if rep == 0:
                    v.wait_ge(s_kf, 16)
                    v.wait_ge(s_cons, 16)
                    v.memset(ones_t, 1.0)
                v.tensor_scalar_mul(lk_t[:, bsl(0)], kf_t[:, bsl(0)],
                                    col(lin_c(0)))
                v.tensor_scalar_mul(lk_t[:, bsl(1)], kf_t[:, bsl(1)],
                                    col(lin_c(1))).then_inc(s_lin, 1)
                # D1 = 2*cos(w0 x) (in place over the ACT cos output)
                v.wait_ge(s_b, 4 * rep + 2)
                v.tensor_scalar_mul(D1k, D1k, 2.0).then_inc(s_d, 1)
                kfold(v, 1, 0, s_f0)
                # prime the bh0 k chain while ACT finishes the q bases
                for n in (2, 3, 4):
                    kstep(v, n, bsl(0))
                    kfold(v, n, 0, s_f0)
                v.wait_ge(s_b, 4 * rep + 4)
                v.tensor_scalar_mul(D1q, D1q, 2.0).then_inc(s_d, 1)
                # interleave q chain with the rest of the bh0 k chain
                kq = 5
                for n in range(2, NH + 1):
                    v.tensor_tensor(Tq[n], D1q, Tq[n - 1], Alu.mult)
                    if n == 2:
                        ins = v.tensor_scalar(Tq[n], Tq[n], col(T0COL), None,
                                              Alu.subtract)
                    else:
                        ins = v.tensor_tensor(Tq[n], Tq[n], Tq[n - 2],
                                              Alu.subtract)
                    ins.then_inc(s_q, 1)
                    if kq <= NH:
                        kstep(v, kq, bsl(0))
                        kfold(v, kq, 0, s_f0)
                        kq += 1
                # last two bh1 folds here so ACT is free for the exps
                v.wait_ge(s_f1, rep * NH + NH - 2)
                v.wait_ge(s_k1, rep * (NH - 1) + NH - 2)
                kfold(v, NH - 1, 1, s_f1)
                v.wait_ge(s_k1, rep * (NH - 1) + NH - 1)
                kfold(v, NH, 1, s_f1)
                # masked softmax prep + finish
                if rep == 0:
                    v.wait_ge(s_mask, 16 * NSM)
                for idx in range(NSM):
                    v.wait_ge(s_mm, rep * NSM + idx + 1)
                    if rep >= 1:
                        v.wait_ge(s_E, (rep - 1) * NSM + idx + 1)
                    v.scalar_tensor_tensor(
                        sc_t[idx], mask_t[idx], -1e30, sc_t[idx],
                        Alu.mult, Alu.add).then_inc(s_scm, 1)
                for idx in range(NSM):
                    v.wait_ge(s_E, rep * NSM + idx + 1)
                    if rep >= 1 and idx == 0:
                        v.wait_ge(s_out, 16 * NSM * rep)
                    v.reciprocal(r_t[idx], sums_t[idx])
                    v.drain()  # r is a scalar operand of the next op
                    v.tensor_scalar_mul(W_t[idx], E_t[idx],
                                        r_t[idx]).then_inc(s_W, 1)

        @block.gpsimd
        def _(g):
            # bh1 k chain on the otherwise-idle Pool engine (folds on ACT)
            for rep in range(reps):
                if rep >= 1:
                    g.wait_ge(s_mm, NSM * rep)
                g.wait_ge(s_b, 4 * rep + 1)
                g.wait_ge(s_d, 2 * rep + 1)
                for n in range(2, NH + 1):
                    kstep(g, n, bsl(1)).then_inc(s_k1, 1)

        @block.tensor
        def _(t):
            for rep in range(reps):
                # per-n interleave matched to production rate; within a
                # harmonic bh0 goes first so its tiles retire first
                t.wait_ge(s_lin, rep + 1)
                for idx in range(NSM):
                    bh, ib = divmod(idx, NIB)
                    if rep >= 1:
                        # PSUM bank reusable once prior exp consumed it
                        t.wait_ge(s_E, (rep - 1) * NSM + idx + 1)
                    t.matmul(sc_t[idx], ones_t[:, 0:128],
                             lk_t[:, bsl(bh)], start=True, stop=False)
                for n in range(1, NH + 1):
                    if n == 1:
                        t.wait_ge(s_b, 4 * rep + 3)   # Tq[1] written by ACT
                    else:
                        t.wait_ge(s_q, rep * (NH - 1) + n - 1)
                    for bh, s_f in ((0, s_f0), (1, s_f1)):
                        t.wait_ge(s_f, rep * NH + n)
                        for ib in range(NIB):
                            idx = bh * NIB + ib
                            ins = t.matmul(
                                sc_t[idx],
                                Tq[n][:, bh * LQ + ib * 128:
                                      bh * LQ + (ib + 1) * 128],
                                Fk[n][:, bsl(bh)],
                                start=False, stop=(n == NH))
                            if n == NH:
                                ins.then_inc(s_mm, 1)

    return nc


def _shard(q, k, a, mask):
    qf = q.reshape(B * H, LQ, D)
    kf = k.reshape(B * H, LK, D)
    mf = mask.reshape(B * H, LQ, LK)
    af = np.ascontiguousarray(
        np.broadcast_to(a.reshape(1, H, D), (B, H, D))).reshape(B * H, D)
    in_maps = []
    for c in range(NCORES):
        sl = slice(NBH * c, NBH * (c + 1))
        # features layout: [partition (cs,d) = 128, free (bh, x) = 768],
        # x replicated across the cs halves
        qT = qf[sl].transpose(0, 2, 1)           # [NBH, 64, LQ]
        kT = kf[sl].transpose(0, 2, 1)
        qF = np.zeros((128, NBH, LQ), np.float32)
        kF = np.zeros((128, NBH, LQ), np.float32)
        for half in range(2):
            qF[half * 64:(half + 1) * 64] = qT.transpose(1, 0, 2)
            kF[half * 64:(half + 1) * 64] = kT.transpose(1, 0, 2)
        cons = np.zeros((128, NCOL), np.float32)
        cons[0:64, 0] = np.pi / 2                # T1 bias: cos top, sin bottom
        cons[:, 1] = np.pi / 2                   # C1 bias: cos everywhere
        cons[0:64, 2] = 1.0                      # T0 = (1 | 0)
        sign = np.concatenate([np.ones(64), -np.ones(64)]).astype(np.float32)
        for bh in range(NBH):
            ac = af[NBH * c + bh]
            ad = np.concatenate([ac, ac]).astype(np.float32)   # per (cs,d)
            for n in range(1, NH + 1):
                cons[:, 3 + 2 * (n - 1) + bh] = sign * ad * np.float32(ALPHA[n])
            cons[:, 3 + 2 * NH + bh] = ad / 4.0
        masku8 = np.ascontiguousarray(mf[sl]).astype(np.uint8)
        in_maps.append(dict(qf=qF.reshape(128, FW), kf=kF.reshape(128, FW),
                            cons=cons, masku8=masku8))
    return in_maps


def kernel(q, k, attention, mask):
    global _built
    q = np.asarray(q, np.float32)
    k = np.asarray(k, np.float32)
    a = np.asarray(attention, np.float32)
    mask = np.asarray(mask).astype(bool)

    in_maps = _shard(q, k, a, mask)
    if _built is None:
        _built = _build()
    res = run_bass_kernel_spmd(_built, in_maps, core_ids=list(range(NCORES)))
    w = np.stack([res.results[c]["w"] for c in range(NCORES)], axis=0)
    return w.reshape(B, H, LQ, LK).astype(np.float32)


# revision 18
# speedup vs baseline: 4.8646x; 1.1667x over previous
"""GATv2 attention-weights kernel for 8 Trainium2 NeuronCores.

Problem (per full input):
    q: (2, 8, 384, 64) f32, k: (2, 8, 384, 64) f32,
    attention: (1, 8, 1, 1, 64) f32, mask: (2, 8, 384, 384) bool
    scores[b,h,i,j] = sum_d silu(q[b,h,i,d] + k[b,h,j,d]) * attention[h,d]
    out = softmax over j with mask (-inf before, 0 after)

Sharding: data-parallel over the 16 (b,h) pairs, 2 per core.

Algorithm (separable-score trick): silu(s) = s/2 + g(s) with g even;
g(s) ~= sum_n alpha_n cos(n*w0*s) (least-squares fit, w0 = pi/12, n<=NH).
Each cosine term splits exactly:
    cos(n w0 (q+k)) = cos(n w0 q)cos(n w0 k) - sin(n w0 q)sin(n w0 k)
so the (i,j) score matrix becomes a plain matmul over (d, harmonic):
    score_ij = sum_d a_d k_jd/2                          (q-part is j-const,
                                                          softmax-invariant)
             + sum_n alpha_n sum_d a_d cos(n w0 (q+k))   (via PE)
Per-core device pipeline:
  - ACT: base features T1 = (cos(w0 x) | sin(w0 x)) packed on the partition
    axis as (cs, d) via one Sin activation with a per-partition bias column
    (pi/2 top half / 0 bottom); arguments stay inside the Sin table's valid
    range [-pi, pi].  Higher harmonics CANNOT use the Sin table (range), so:
  - DVE: harmonic chains T_{n+1} = D1 . T_n - T_{n-1} (D1 = 2*cos(w0 x)
    replicated over both cs halves) in fp16 (2x DVE mode).  DVE runs the
    q chain (768 wide, both bh) and the bh0 k chain; the bh1 k chain runs
    on the otherwise-idle Pool/GPSIMD engine.  k-features are folded by
    +-a_d*alpha_n per-partition columns (fp16 4x mode on DVE; most bh1
    folds on ACT via Copy-with-scale).
  - PE: one 128-wide fp16 matmul per (harmonic, bh, i-block) accumulating
    scores into 6 PSUM tiles, plus a rank-1 term for the linear part
    (ones x a_d k/4 over all 128 partitions) which opens each group.
  - Masked softmax: (mask*-1e30)+scores on Pool (free after its chain),
    exp with fused row-sum on ACT, reciprocal + scale on DVE.  Scores are
    bounded (|s|<8): exp cannot overflow, no row-max pass needed.
"""

import numpy as np
from contextlib import ExitStack

import concourse.bass as bass
from concourse import mybir
from concourse.bass_utils import run_bass_kernel_spmd

B, H, LQ, LK, D = 2, 8, 384, 384, 64
NCORES = 8
NBH = (B * H) // NCORES        # 2 (b,h) pairs per core
NIB = LQ // 128                # 3 i-blocks
NSM = NBH * NIB                # 6 softmax tiles
FW = NBH * LQ                  # 768 free width (bh-packed)

NH = 8                         # harmonics
W0 = 0.2617993877991494        # pi/12
ALPHAS = {
    8: [2.908628417, -2.494822702, -0.09452154696, -0.2374207151,
        -0.02201108313, -0.04341576159, -0.005720147384, -0.006775574468,
        -0.002421780018],
    10: [2.914250703, -2.50581741, -0.08424785112, -0.2465759164,
         -0.01425584389, -0.04962360035, -0.00106553002, -0.009994115834,
         -0.0004242872451, -0.0015243423, -0.0004320355077],
}
ALPHA = ALPHAS[NH]
NCOL = 3 + 2 * NH + NBH        # consts columns

_f32 = mybir.dt.float32
_f16 = mybir.dt.float16
_u8 = mybir.dt.uint8

_built = None  # cache across calls


def _build(reps=1):
    # reps > 1 unrolls the whole computation N times inside one program
    # (used only for steady-state timing; the grading path uses reps=1).
    AF = mybir.ActivationFunctionType
    Alu = mybir.AluOpType

    nc = bass.Bass("TRN2", target_bir_lowering=False, debug=False,
                   num_devices=NCORES)

    qf_d = nc.dram_tensor("qf", [128, FW], _f32, kind="ExternalInput").ap()
    kf_d = nc.dram_tensor("kf", [128, FW], _f32, kind="ExternalInput").ap()
    cons_d = nc.dram_tensor("cons", [128, NCOL], _f32, kind="ExternalInput").ap()
    mask_d = nc.dram_tensor("masku8", [NBH, LQ, LK], _u8, kind="ExternalInput").ap()
    w_d = nc.dram_tensor("w", [NBH, LQ, LK], _f32, kind="ExternalOutput").ap()

    qf_t = nc.alloc_sbuf_tensor("qf_t", [128, FW], _f32).ap()
    kf_t = nc.alloc_sbuf_tensor("kf_t", [128, FW], _f32).ap()
    cons_t = nc.alloc_sbuf_tensor("cons_t", [128, NCOL], _f32).ap()
    mask_t = [nc.alloc_sbuf_tensor(f"mask_t{i}", [128, LK], _u8).ap()
              for i in range(NSM)]
    # feature tensors (fp16), harmonic index 1..NH
    Tq = [None] + [nc.alloc_sbuf_tensor(f"Tq{n}", [128, FW], _f16).ap()
                   for n in range(1, NH + 1)]
    Tk = [None] + [nc.alloc_sbuf_tensor(f"Tk{n}", [128, FW], _f16).ap()
                   for n in range(1, NH + 1)]
    Fk = [None] + [nc.alloc_sbuf_tensor(f"Fk{n}", [128, FW], _f16).ap()
                   for n in range(1, NH + 1)]
    D1q = nc.alloc_sbuf_tensor("D1q", [128, FW], _f16).ap()
    D1k = nc.alloc_sbuf_tensor("D1k", [128, FW], _f16).ap()
    lk_t = nc.alloc_sbuf_tensor("lk_t", [128, FW], _f16).ap()
    ones_t = nc.alloc_sbuf_tensor("ones_t", [128, 128], _f16).ap()
    E_t = [nc.alloc_sbuf_tensor(f"E{i}", [128, LK], _f32).ap()
           for i in range(NSM)]
    Wb_t = nc.alloc_sbuf_tensor("Wb", [128, NSM * LK], _f32).ap()
    W_t = [Wb_t[:, i * LK:(i + 1) * LK] for i in range(NSM)]
    sums_t = [nc.alloc_sbuf_tensor(f"sums{i}", [128, 1], _f32).ap()
              for i in range(NSM)]
    r_t = [nc.alloc_sbuf_tensor(f"r{i}", [128, 1], _f32).ap()
           for i in range(NSM)]
    sc_t = [nc.alloc_psum_tensor(f"sc{i}", [128, LK], _f32).ap()
            for i in range(NSM)]

    col = lambda c: cons_t[:, c:c + 1]
    BIAS_T1, BIAS_C1, T0COL = 0, 1, 2
    fold_c = lambda n, bh: 3 + 2 * (n - 1) + bh
    lin_c = lambda bh: 3 + 2 * NH + bh
    bsl = lambda bh: slice(bh * LQ, (bh + 1) * LQ)   # free-slice of one bh

    with ExitStack() as ctx:
        s_cons = ctx.enter_context(nc.semaphore("s_cons"))
        s_qf = ctx.enter_context(nc.semaphore("s_qf"))
        s_kf = ctx.enter_context(nc.semaphore("s_kf"))
        s_mask = ctx.enter_context(nc.semaphore("s_mask"))
        s_b = ctx.enter_context(nc.semaphore("s_b"))      # ACT bases, 4/rep
        s_d = ctx.enter_context(nc.semaphore("s_d"))      # D1k ready, 1/rep
        s_q = ctx.enter_context(nc.semaphore("s_q"))      # q chain, NH-1/rep
        s_k1 = ctx.enter_context(nc.semaphore("s_k1"))    # Pool k1, NH-1/rep
        s_f0 = ctx.enter_context(nc.semaphore("s_f0"))    # k folds bh0, NH/rep
        s_f1 = ctx.enter_context(nc.semaphore("s_f1"))    # k folds bh1, NH/rep
        s_lin = ctx.enter_context(nc.semaphore("s_lin"))  # lk+ones, 1/rep
        s_mm = ctx.enter_context(nc.semaphore("s_mm"))    # PE tile stops, 6/rep
        s_scm = ctx.enter_context(nc.semaphore("s_scm"))
        s_E = ctx.enter_context(nc.semaphore("s_E"))
        s_W = ctx.enter_context(nc.semaphore("s_W"))
        s_out = ctx.enter_context(nc.semaphore("s_out"))
        block = ctx.enter_context(nc.Block())

        # k chain step on an engine `v` for one bh slice; returns last instr
        def kstep(v, n, sl):
            v.tensor_tensor(Tk[n][:, sl], D1k[:, sl], Tk[n - 1][:, sl],
                            Alu.mult)
            if n == 2:
                return v.tensor_scalar(Tk[n][:, sl], Tk[n][:, sl], col(T0COL),
                                       None, Alu.subtract)
            return v.tensor_tensor(Tk[n][:, sl], Tk[n][:, sl],
                                   Tk[n - 2][:, sl], Alu.subtract)

        def kfold(v, n, bh, s_f):
            sl = bsl(bh)
            v.tensor_scalar_mul(Fk[n][:, sl], Tk[n][:, sl],
                                col(fold_c(n, bh))).then_inc(s_f, 1)

        @block.sync
        def _(sp):
            sp.dma_start(out=kf_t, in_=kf_d).then_inc(s_kf, 16)
            sp.dma_start(out=cons_t, in_=cons_d).then_inc(s_cons, 16)
            sp.dma_start(out=qf_t, in_=qf_d).then_inc(s_qf, 16)
            for idx in range(NSM):
                bh, ib = divmod(idx, NIB)
                sp.dma_start(out=mask_t[idx],
                             in_=mask_d[bh, ib * 128:(ib + 1) * 128, :]
                             ).then_inc(s_mask, 16)
            for rep in range(reps):
                for idx in (0, 1, 2, 3, 4):
                    bh, ib = divmod(idx, NIB)
                    sp.wait_ge(s_W, rep * NSM + idx + 1)
                    sp.dma_start(out=w_d[bh, ib * 128:(ib + 1) * 128, :],
                                 in_=W_t[idx]).then_inc(s_out, 16)
            sp.wait_ge(s_out, 16 * NSM * reps)

        @block.scalar
        def _(a):
            a.wait_ge(s_cons, 16)
            a.wait_ge(s_kf, 16)
            for rep in range(reps):
                if rep >= 1:
                    # feature/base tensors reusable once all prior-rep
                    # matmuls retired
                    a.wait_ge(s_mm, NSM * rep)
                a.activation(D1k, kf_t, AF.Sin,
                             bias=col(BIAS_C1), scale=W0).then_inc(s_b, 1)
                a.activation(Tk[1], kf_t, AF.Sin,
                             bias=col(BIAS_T1), scale=W0).then_inc(s_b, 1)
                if rep == 0:
                    a.wait_ge(s_qf, 16)
                a.activation(Tq[1], qf_t, AF.Sin,
                             bias=col(BIAS_T1), scale=W0).then_inc(s_b, 1)
                a.activation(D1q, qf_t, AF.Sin,
                             bias=col(BIAS_C1), scale=W0).then_inc(s_b, 1)

                # bh1 folds n=1..NH-2 (Copy with per-partition scale);
                # the last two are on DVE so exps start promptly here
                for n in range(1, NH - 1):
                    if n >= 2:
                        a.wait_ge(s_k1, rep * (NH - 1) + n - 1)
                    sl = bsl(1)
                    a.activation(Fk[n][:, sl], Tk[n][:, sl], AF.Copy,
                                 scale=col(fold_c(n, 1))).then_inc(s_f1, 1)

                for idx in range(NSM):
                    a.wait_ge(s_scm, rep * NSM + idx + 1)
                    if rep >= 1:
                        a.wait_ge(s_W, (rep - 1) * NSM + idx + 1)
                    a.activation(E_t[idx], sc_t[idx], AF.Exp,
                                 accum_out=sums_t[idx]).then_inc(s_E, 1)
                for idx in (5,):
                    bh, ib = divmod(idx, NIB)
                    a.wait_ge(s_W, rep * NSM + idx + 1)
                    a.dma_start(out=w_d[bh, ib * 128:(ib + 1) * 128, :],
                                in_=W_t[idx]).then_inc(s_out, 16)

        @block.vector
        def _(v):
            for rep in range(reps):
                if rep >= 1:
                    v.wait_ge(s_mm, NSM * rep)
                if rep == 0:
                    v.wait_ge(s_kf, 16)
                    v.wait_ge(s_cons, 16)
                    v.memset(ones_t, 1.0)
                v.tensor_scalar_mul(lk_t[:, bsl(0)], kf_t[:, bsl(0)],
                                    col(lin_c(0)))
                v.tensor_scalar_mul(lk_t[:, bsl(1)], kf_t[:, bsl(1)],
                                    col(lin_c(1))).then_inc(s_lin, 1)
                # D1 = 2*cos(w0 x) in place
                v.wait_ge(s_b, 4 * rep + 1)
                v.tensor_scalar_mul(D1k, D1k, 2.0).then_inc(s_d, 1)
                v.wait_ge(s_b, 4 * rep + 2)
                kfold(v, 1, 0, s_f0)
                # prime the bh0 k chain while ACT finishes the q bases
                for n in (2, 3, 4):
                    kstep(v, n, bsl(0))
                    kfold(v, n, 0, s_f0)
                v.wait_ge(s_b, 4 * rep + 4)
                v.tensor_scalar_mul(D1q, D1q, 2.0)
                # interleave q chain with the rest of the bh0 k chain
                kq = 5
                for n in range(2, NH + 1):
                    v.tensor_tensor(Tq[n], D1q, Tq[n - 1], Alu.mult)
                    if n == 2:
                        ins = v.tensor_scalar(Tq[n], Tq[n], col(T0COL), None,
                                              Alu.subtract)
                    else:
                        ins = v.tensor_tensor(Tq[n], Tq[n], Tq[n - 2],
                                              Alu.subtract)
                    ins.then_inc(s_q, 1)
                    if kq <= NH:
                        kstep(v, kq, bsl(0))
                        kfold(v, kq, 0, s_f0)
                        kq += 1
                # last two bh1 folds here so ACT is free for the exps
                v.wait_ge(s_f1, rep * NH + NH - 2)
                v.wait_ge(s_k1, rep * (NH - 1) + NH - 2)
                kfold(v, NH - 1, 1, s_f1)
                v.wait_ge(s_k1, rep * (NH - 1) + NH - 1)
                kfold(v, NH, 1, s_f1)
                if rep == 0:
                    v.wait_ge(s_mask, 16 * NSM)
                for idx in range(NSM):
                    v.wait_ge(s_mm, rep * NSM + idx + 1)
                    if rep >= 1:
                        v.wait_ge(s_E, (rep - 1) * NSM + idx + 1)
                    v.scalar_tensor_tensor(
                        sc_t[idx], mask_t[idx], -1e30, sc_t[idx],
                        Alu.mult, Alu.add).then_inc(s_scm, 1)
                for idx in range(NSM):
                    v.wait_ge(s_E, rep * NSM + idx + 1)
                    if rep >= 1 and idx == 0:
                        v.wait_ge(s_out, 16 * NSM * rep)
                    v.reciprocal(r_t[idx], sums_t[idx])
                    v.drain()  # r is a scalar operand of the next op
                    v.tensor_scalar_mul(W_t[idx], E_t[idx],
                                        r_t[idx]).then_inc(s_W, 1)

        @block.gpsimd
        def _(g):
            # bh1 k chain on the otherwise-idle Pool engine, then the
            # mask-add for each retired score tile
            for rep in range(reps):
                if rep >= 1:
                    g.wait_ge(s_mm, NSM * rep)
                g.wait_ge(s_b, 4 * rep + 2)
                g.wait_ge(s_d, rep + 1)
                for n in range(2, NH + 1):
                    kstep(g, n, bsl(1)).then_inc(s_k1, 1)

        @block.tensor
        def _(t):
            for rep in range(reps):
                # rank-1 linear term opens each tile's accumulation group
                t.wait_ge(s_lin, rep + 1)
                for idx in range(NSM):
                    bh, ib = divmod(idx, NIB)
                    if rep >= 1:
                        # PSUM bank reusable once prior rep's exp consumed it
                        t.wait_ge(s_E, (rep - 1) * NSM + idx + 1)
                    t.matmul(sc_t[idx], ones_t[:, 0:128],
                             lk_t[:, bsl(bh)], start=True, stop=False)
                for n in range(1, NH + 1):
                    if n == 1:
                        t.wait_ge(s_b, 4 * rep + 3)   # Tq[1] written by ACT
                    else:
                        t.wait_ge(s_q, rep * (NH - 1) + n - 1)
                    for bh, s_f in ((0, s_f0), (1, s_f1)):
                        t.wait_ge(s_f, rep * NH + n)
                        for ib in range(NIB):
                            idx = bh * NIB + ib
                            ins = t.matmul(
                                sc_t[idx],
                                Tq[n][:, bh * LQ + ib * 128:
                                      bh * LQ + (ib + 1) * 128],
                                Fk[n][:, bsl(bh)],
                                start=False, stop=(n == NH))
                            if n == NH:
                                ins.then_inc(s_mm, 1)

    return nc


def _shard(q, k, a, mask):
    qf = q.reshape(B * H, LQ, D)
    kf = k.reshape(B * H, LK, D)
    mf = mask.reshape(B * H, LQ, LK)
    af = np.ascontiguousarray(
        np.broadcast_to(a.reshape(1, H, D), (B, H, D))).reshape(B * H, D)
    in_maps = []
    for c in range(NCORES):
        sl = slice(NBH * c, NBH * (c + 1))
        # features layout: [partition (cs,d) = 128, free (bh, x) = 768],
        # x replicated across the cs halves
        qT = qf[sl].transpose(0, 2, 1)           # [NBH, 64, LQ]
        kT = kf[sl].transpose(0, 2, 1)
        qF = np.zeros((128, NBH, LQ), np.float32)
        kF = np.zeros((128, NBH, LQ), np.float32)
        for half in range(2):
            qF[half * 64:(half + 1) * 64] = qT.transpose(1, 0, 2)
            kF[half * 64:(half + 1) * 64] = kT.transpose(1, 0, 2)
        cons = np.zeros((128, NCOL), np.float32)
        cons[0:64, 0] = np.pi / 2                # T1 bias: cos top, sin bottom
        cons[:, 1] = np.pi / 2                   # C1 bias: cos everywhere
        cons[0:64, 2] = 1.0                      # T0 = (1 | 0)
        sign = np.concatenate([np.ones(64), -np.ones(64)]).astype(np.float32)
        for bh in range(NBH):
            ac = af[NBH * c + bh]
            ad = np.concatenate([ac, ac]).astype(np.float32)   # per (cs,d)
            for n in range(1, NH + 1):
                cons[:, 3 + 2 * (n - 1) + bh] = sign * ad * np.float32(ALPHA[n])
            cons[:, 3 + 2 * NH + bh] = ad / 4.0
        masku8 = np.ascontiguousarray(mf[sl]).astype(np.uint8)
        in_maps.append(dict(qf=qF.reshape(128, FW), kf=kF.reshape(128, FW),
                            cons=cons, masku8=masku8))
    return in_maps


def kernel(q, k, attention, mask):
    global _built
    q = np.asarray(q, np.float32)
    k = np.asarray(k, np.float32)
    a = np.asarray(attention, np.float32)
    mask = np.asarray(mask).astype(bool)

    in_maps = _shard(q, k, a, mask)
    if _built is None:
        _built = _build()
    res = run_bass_kernel_spmd(_built, in_maps, core_ids=list(range(NCORES)))
    w = np.stack([res.results[c]["w"] for c in range(NCORES)], axis=0)
    return w.reshape(B, H, LQ, LK).astype(np.float32)
